# revision 1
# baseline (speedup 1.0000x reference)
"""MiniBatchDiscrimination kernel, v5: relu elementwise on three engines
+ 4-way column-tiled PE streams + paired-row Exp.

Math per core (row block of 64 i's x FD=320 j columns spanning 5 blocks):
  Mt[(o,k), j] = M^T in bf16 (16 partition-tiles of 128 = 4 o x 32 k),
  computed on PE from T and x^T.
  For each i: relu tiles R_t = max(Mt_t - Mt_t[:, i], 0) on DVE (11) and
    GpSimd (2); abs tiles |Mt_t - Mt_t[:, i]| on ACT (3) via
    activation(Abs, scale=-1, bias).  The scalar column is the bf16 Mt
    value recast to fp32 (mcol), so the diagonal difference is exactly 0.
  D[o, j] = 2*sum_k relu + sum_k |d| - (S_j - S_i): the k-reduction is 16
    matmuls per i with selection weights (2.0 relu / 1.0 abs tiles, 4
    nonzero output rows each); -S_j lands first via one dmap matmul per
    PAIR covering all partitions; +S_i rides the Exp bias (Sneg2).
  Two i's share one PSUM bank: even i -> partitions 0:64, odd -> 64:128,
  each split again into o-halves -> FOUR concurrent column-tiled PE
  streams (tile_position (0,0)/(0,32)/(0,64)/(0,96), 32-wide weights).
  The bank is reset by a tiny start=True matmul into its top 2 columns
  (start marks the whole 2KB zero region pending-zero), so the dmap
  matmul overwrites and the streams accumulate with start=False.
  One Exp per PAIR of i's: activation [128, 320] with accum_out giving
  both rowsums; column-sum partials accumulate in a persistent PSUM bank
  via one matmul per two pairs (a DVE add folds two exp tiles first).

Sharding (unchanged from v4): symmetric-pair blocks, 5 column blocks per
core (cores 4-7 carry one poisoned block); host adds row- and mirrored
column-sums and subtracts the self-similarity 1.
"""

import numpy as np
import ml_dtypes
from contextlib import ExitStack

BATCH, IN_FEAT, OUT_FEAT, KERNEL_DIM = 512, 512, 64, 32
N_CORES = 8
ROWB = BATCH // N_CORES          # 64 rows of i per core
NPAIR = ROWB // 2                # 32 exp/psum groups
OK = OUT_FEAT * KERNEL_DIM       # 2048 flattened (o,k)
NT = OK // 128                   # 16 partition-tiles of (o,k)
NBLK = 5                         # column blocks per core
FD = NBLK * 64                   # 320
POISON = 1.0e4

SELW = 32                        # per-tile weight width (o-half streams)
ACT_TILES = (5, 10, 13)          # elementwise tiles computed on ACT as Abs
POOL_TILES = (2, 7)              # elementwise tiles computed on GpSimd
DVE_BUFS = 48
ACT_BUFS = 20
POOL_BUFS = 12
# NOTE: the walrus ISA rejects abs_max on InstTensorScalarPtr (probed:
# every variant), so the DVE/Pool tiles compute relu(d) (weight 2.0) and
# the missing -d term is restored per pair by one -S_j matmul (dmap)
# plus the +S_i exp bias: sum|d| = 2*sum relu(d) - (S_j - S_i).

_cache = {}


def _build_nc(split_waits=True):
    import concourse.bass as bass
    import concourse.mybir as mybir
    import concourse.tile as tile

    dt = mybir.dt
    AF = mybir.ActivationFunctionType
    OP = mybir.AluOpType

    nc = bass.Bass("TRN2", target_bir_lowering=False, debug=False,
                   num_devices=N_CORES)

    xT_d = nc.dram_tensor("xT", [IN_FEAT, FD], dt.bfloat16, kind="ExternalInput")
    T_d = nc.dram_tensor("Tm", [IN_FEAT, OK], dt.bfloat16, kind="ExternalInput")
    sel_d = nc.dram_tensor("sel", [128, NT * SELW], dt.bfloat16,
                           kind="ExternalInput")
    sel2_d = nc.dram_tensor("sel2", [128, OUT_FEAT], dt.bfloat16,
                            kind="ExternalInput")
    dmap_d = nc.dram_tensor("dmap", [OUT_FEAT, 128], dt.bfloat16,
                            kind="ExternalInput")
    rows_d = nc.dram_tensor("rowS2", [128, NPAIR], dt.float32,
                            kind="ExternalOutput")
    acc_d = nc.dram_tensor("accS", [OUT_FEAT, FD], dt.float32,
                           kind="ExternalOutput")
    # the last pair's exp tile goes out raw; the host folds it into the
    # column sums, keeping the final colsum+copy+DMA off the kernel tail
    eL_d = nc.dram_tensor("eLast", [128, FD], dt.bfloat16,
                          kind="ExternalOutput")

    with tile.TileContext(nc) as tc, ExitStack() as ctx:
        const = ctx.enter_context(tc.tile_pool(name="const", bufs=1))
        mtp = ctx.enter_context(tc.tile_pool(name="mt", bufs=NT))
        psA = ctx.enter_context(
            tc.tile_pool(name="psA", bufs=2, space=bass.MemorySpace.PSUM))
        psWp = ctx.enter_context(
            tc.tile_pool(name="psW", bufs=1, space=bass.MemorySpace.PSUM))
        psSp = ctx.enter_context(
            tc.tile_pool(name="psS", bufs=1, space=bass.MemorySpace.PSUM))
        psD = ctx.enter_context(
            tc.tile_pool(name="psD", bufs=3, space=bass.MemorySpace.PSUM))
        psC = ctx.enter_context(
            tc.tile_pool(name="psC", bufs=1, space=bass.MemorySpace.PSUM))
        workV = ctx.enter_context(tc.tile_pool(name="workV", bufs=DVE_BUFS))
        workP = ctx.enter_context(tc.tile_pool(name="workP", bufs=POOL_BUFS))
        # ACT-written tiles (Abs elementwise + Exp outputs) share one pool:
        # the WAW chain keeps ACT's scheduled order near program order
        ep = ctx.enter_context(tc.tile_pool(name="e", bufs=ACT_BUFS))

        # inputs split between the two HWDGE queues (SP + ACT) and
        # interleaved T/xT so the first M-setup matmuls start early
        # T chunks on the SP queue, xT/sel on the ACT queue: the DGE
        # round-robins the queues, interleaving each T chunk with the xT
        # it is contracted against
        Tsb, xTsb = [], []
        for kc in range(4):
            t_ = const.tile([128, OK], dt.bfloat16, tag=f"T{kc}")
            nc.sync.dma_start(t_[:], T_d[kc * 128:(kc + 1) * 128, :])
            Tsb.append(t_)
            x_ = const.tile([128, FD], dt.bfloat16, tag=f"x{kc}")
            nc.scalar.dma_start(x_[:], xT_d[kc * 128:(kc + 1) * 128, :])
            xTsb.append(x_)
        sel = const.tile([128, NT * SELW], dt.bfloat16, tag="sel")
        nc.sync.dma_start(sel[:], sel_d[:])
        sel2 = const.tile([128, OUT_FEAT], dt.bfloat16, tag="sel2")
        nc.sync.dma_start(sel2[:], sel2_d[:])
        dmap = const.tile([OUT_FEAT, 128], dt.bfloat16, tag="dmap")
        nc.sync.dma_start(dmap[:], dmap_d[:])
        # 1x128 zero weight: a K=1 matmul with it writes 0 to a whole PSUM
        # bank, resetting values + has_written in one cheap PE instruction
        zeroW = const.tile([1, 128], dt.bfloat16, tag="zeroW")
        nc.vector.memset(zeroW[:], 0.0)
        # two tiles so the first half's DMA only waits on exps 0..15
        rowS2a = const.tile([128, NPAIR // 2], dt.float32, tag="rowS2a")
        rowS2b = const.tile([128, NPAIR // 2], dt.float32, tag="rowS2b")
        accS = const.tile([OUT_FEAT, FD], dt.float32, tag="accS")
        # scalar columns: the *rounded* bf16 values recast to fp32 so the
        # diagonal difference is exactly zero.  One tile per consuming
        # engine, written BY that engine, so each absdiff's scalar read
        # needs no cross-engine wait
        n_dve = NT - len(ACT_TILES) - len(POOL_TILES)
        mcolV = const.tile([128, n_dve * ROWB], dt.float32, tag="mcolV")
        mcolA = const.tile([128, len(ACT_TILES) * ROWB], dt.float32,
                           tag="mcolA")
        mcolP = const.tile([128, len(POOL_TILES) * ROWB], dt.float32,
                           tag="mcolP")
        mcol_of = {}
        for t in range(NT):
            if t in ACT_TILES:
                mcol_of[t] = (mcolA, ACT_TILES.index(t))
            elif t in POOL_TILES:
                mcol_of[t] = (mcolP, POOL_TILES.index(t))
            else:
                dv = [u for u in range(NT)
                      if u not in ACT_TILES and u not in POOL_TILES]
                mcol_of[t] = (mcolV, dv.index(t))

        # warm the PE clock (HAM / p-state) during the input DMA window
        # with zero matmuls on the already-memset zeroW tile
        psW = psWp.tile([128, 128], dt.float32, tag="psW")
        for w in range(50):
            nc.tensor.matmul(psW[:], zeroW[:], zeroW[:],
                             start=True, stop=True)

        # Mt tiles: Mt[(o,k), j], tile t holds o in [4t, 4t+4), all k.
        # PSUM->SBUF bf16 copies split between DVE and ACT.
        mts = [None] * NT
        for t in [v for p in range(NT // 2) for v in (p, p + NT // 2)]:
            ps = psA.tile([128, FD], dt.float32)
            for kc in range(4):
                nc.tensor.matmul(ps[:],
                                 Tsb[kc][:, t * 128:(t + 1) * 128],
                                 xTsb[kc][:],
                                 start=(kc == 0), stop=(kc == 3))
            mt_t = mtp.tile([128, FD], dt.bfloat16, tag="mt")
            if t % 2 == 0:
                nc.vector.tensor_copy(mt_t[:], ps[:])
            else:
                nc.scalar.copy(mt_t[:], ps[:])
            mc, ci = mcol_of[t]
            dst = mc[:, ci * ROWB:(ci + 1) * ROWB]
            if t in ACT_TILES:
                nc.scalar.copy(dst, mt_t[:, 0:ROWB])
            elif t in POOL_TILES:
                nc.gpsimd.tensor_copy(dst, mt_t[:, 0:ROWB])
            else:
                nc.vector.tensor_copy(dst, mt_t[:, 0:ROWB])
            mts[t] = mt_t

        lo_tiles = [t for t in range(NT // 2)]          # o in [0, 32)
        hi_tiles = [t for t in range(NT // 2, NT)]      # o in [32, 64)

        # S[o, j] = sum_k Mt[(o,k), j] over the relu tiles (selS is zero on
        # the ACT tiles' o's), in bf16 so the exp bias cancels the -S_j
        # matmul exactly on the diagonal
        r_tiles = [t for t in range(NT) if t not in ACT_TILES]
        psS = psSp.tile([OUT_FEAT, 512], dt.float32, tag="psS")
        nc.tensor.matmul(psS[:, 510:512], zeroW[0:1, 0:OUT_FEAT],
                         sel[0:1, 0:2], start=True, stop=True)
        for t in r_tiles:
            oh = 0 if t < NT // 2 else 32
            nc.tensor.matmul(psS[oh:oh + 32, 0:FD],
                             sel[:, t * SELW:(t + 1) * SELW], mts[t][:],
                             start=False, stop=False, skip_group_check=True,
                             tile_position=(0, oh))
        # sel carries the relu weight 2.0; halve while converting to bf16
        # (on ACT: DVE is the busier engine during the pipeline fill)
        S_bf = const.tile([OUT_FEAT, FD], dt.bfloat16, tag="S_bf")
        nc.scalar.activation(S_bf[:], psS[:, 0:FD], AF.Copy, scale=0.5)
        # exp bias: Sneg2[p, m] = -S[o(p), 2m + (p // 64)]
        Sneg2 = const.tile([128, NPAIR], dt.float32, tag="Sneg2")
        nc.scalar.activation(Sneg2[0:OUT_FEAT, :], S_bf[:, 0:2 * NPAIR:2],
                             AF.Copy, scale=-1.0)
        nc.scalar.activation(Sneg2[OUT_FEAT:128, :], S_bf[:, 1:2 * NPAIR:2],
                             AF.Copy, scale=-1.0)

        def absdiff(t, i, name):
            mc, ci = mcol_of[t]
            sc = mc[:, ci * ROWB + i: ci * ROWB + i + 1]
            if t in ACT_TILES:
                ad_t = ep.tile([128, FD], dt.bfloat16, tag="e", name=name)
                nc.scalar.activation(ad_t[:], mts[t][:], AF.Abs,
                                     bias=sc, scale=-1.0)
            elif t in POOL_TILES:
                ad_t = workP.tile([128, FD], dt.bfloat16, tag="adP",
                                  name=name)
                nc.gpsimd.tensor_scalar(ad_t[:], mts[t][:], sc, 0.0,
                                        op0=OP.subtract, op1=OP.max)
            else:
                ad_t = workV.tile([128, FD], dt.bfloat16, tag="adV",
                                  name=name)
                nc.vector.tensor_scalar(ad_t[:], mts[t][:], sc, 0.0,
                                        op0=OP.subtract, op1=OP.max)
            return ad_t

        psc = psC.tile([OUT_FEAT, FD], dt.float32, tag="psc")

        def exp_pair(psd, m, accum=True):
            e_t = ep.tile([128, FD], dt.bfloat16, tag="e", name=f"e_{m}")
            half, col = divmod(m, NPAIR // 2)
            rs = rowS2b if half else rowS2a
            kw = {"accum_out": rs[:, col:col + 1]} if accum else {}
            nc.scalar.activation(e_t[:], psd[:, 0:FD], AF.Exp, scale=-1.0,
                                 bias=Sneg2[:, m:m + 1], **kw)
            return e_t

        def colsum(e_a, e_b, q):
            # fold two pairs' exp tiles on DVE, halving the colsum matmuls
            es = workV.tile([128, FD], dt.bfloat16, tag="adV",
                            name=f"esum_{q}")
            nc.vector.tensor_add(es[:], e_a[:], e_b[:])
            nc.tensor.matmul(psc[:], sel2[:], es[:],
                             start=(q == 0), stop=(q == NPAIR // 2 - 1))

        pending = None           # (psd, m) awaiting exp
        pending_e = []           # e tiles awaiting colsum matmul

        for m in range(NPAIR):
            last = m == NPAIR - 1
            iA, iB = 2 * m, 2 * m + 1
            ads = {}
            for p in range(NT // 2):
                for ih, i in ((0, iA), (1, iB)):
                    for t in (lo_tiles[p], hi_tiles[p]):
                        ads[(t, ih)] = absdiff(t, i, f"ad_{m}_{t}_{ih}")
            if last:
                # emit exp(30) before the last pair's matmuls so e30's
                # colsum can slot into the middle of the stream block
                e_prev = exp_pair(*pending)
                pending = None
            # full-bank tile: columns 0:FD carry D.  The dmap matmul both
            # resets the bank and writes -S_j for all four quarters:
            # start=True marks the whole 2KB zero region pending-zero (its
            # own write then lands as an overwrite), stop=True closes the
            # sim's group so the next pair can start; the streams then
            # accumulate with start=False.
            psd = psD.tile([128, 512], dt.float32, tag="psd",
                           name=f"psd_{m}")
            nc.tensor.matmul(psd[:, 0:FD], dmap[:], S_bf[:],
                             start=True, stop=True)
            # 4 column-tiled streams: array col quarter = 64*i + 32*o_half
            for p in range(NT // 2):
                if last and p == 4:
                    # e30 closes the psc group mid-stream: the accS
                    # copies + DMA then overlap the rest of the pair
                    nc.tensor.matmul(psc[:], sel2[:], e_prev[:],
                                     start=False, stop=True)
                for ih in (0, 1):
                    for oh, tlist in ((0, lo_tiles), (1, hi_tiles)):
                        t = tlist[p]
                        cp = 64 * ih + 32 * oh
                        nc.tensor.matmul(
                            psd[cp:cp + 32, 0:FD],
                            sel[:, t * SELW:(t + 1) * SELW],
                            ads[(t, ih)][:],
                            start=False, stop=False,
                            skip_group_check=True,
                            tile_position=(0, cp))
            if last:
                HF = FD // 2
                nc.vector.tensor_copy(accS[:, 0:HF], psc[:, 0:HF])
                nc.scalar.copy(accS[:, HF:FD], psc[:, HF:FD])
                nc.sync.dma_start(acc_d[:], accS[:])
                # rowsums of exps 16..30; the last pair's come from eLast
                nc.sync.dma_start(rows_d[:, NPAIR // 2:NPAIR - 1],
                                  rowS2b[:, 0:NPAIR // 2 - 1])
            # exp of the PREVIOUS pair: its PE wait is already satisfied,
            # so ACT never stalls; colsum trails two pairs behind
            if pending is not None:
                pending_e.append(exp_pair(*pending))
                if len(pending_e) == 2:
                    colsum(*pending_e, pending[1] // 2)
                    pending_e = []
                if pending[1] == NPAIR // 2 - 1:
                    # first half of the rowsums is final: ship it early
                    nc.sync.dma_start(rows_d[:, 0:NPAIR // 2], rowS2a[:])
            pending = (psd, m)
        # tail: exp31's tile ships raw; the host folds it into the column
        # sums and derives the last two rowsums from it
        e_last = exp_pair(*pending, accum=False)
        assert not pending_e
        nc.scalar.dma_start(eL_d[:], e_last[:])

    if split_waits:
        _split_multiwaits(nc, mybir)
    return nc


def _split_multiwaits(nc, mybir):
    """Walrus on this toolchain encodes at most ONE sync-wait command per
    instruction.  Split any instruction with more waits into a chain of
    single-wait Drain carriers on the same engine, inserted immediately
    before it."""
    n = 0
    for fn in nc.m.functions:
        for bb in fn.blocks:
            new_insts = []
            for inst in bb.instructions:
                si = getattr(inst, "sync_info", None)
                if si is not None and si.on_wait and len(si.on_wait) > 1:
                    waits = list(si.on_wait)
                    for w in waits[:-1]:
                        carrier = mybir.InstDrain(
                            name=f"splitw_{n}", engine=inst.engine,
                            ins=[], outs=[],
                            sync_info=mybir.SyncInfo(on_wait=[w],
                                                     on_update=[]))
                        new_insts.append(carrier)
                        n += 1
                    inst.sync_info = mybir.SyncInfo(
                        on_wait=[waits[-1]], on_update=list(si.on_update))
                new_insts.append(inst)
            if n:
                bb.instructions = new_insts


def _sel_host():
    """Selection weights: tile t's partition group g (o = 4t+g, 32 k's)
    sums into weight column (4t+g) mod 32 of its o-half stream, scaled
    2.0 for relu tiles and 1.0 for the ACT |d| tiles."""
    sel = np.zeros((128, NT * SELW), dtype=np.float32)
    for t in range(NT):
        v = 1.0 if t in ACT_TILES else 2.0
        for g in range(4):
            sel[32 * g:32 * (g + 1), t * SELW + (4 * t + g) % SELW] = v
    return sel.astype(ml_dtypes.bfloat16)


def _dmap_host():
    """-S broadcast: output partition p of a pair bank holds (i-half
    p//64, o = 32*((p//32)%2) + p%32) and receives -S[o, j]."""
    d = np.zeros((OUT_FEAT, 128), dtype=np.float32)
    for p in range(128):
        o = 32 * ((p // 32) % 2) + p % 32
        d[o, p] = -1.0
    return d.astype(ml_dtypes.bfloat16)


def _sel2_host():
    s = np.zeros((128, OUT_FEAT), dtype=np.float32)
    s[:OUT_FEAT, :] = np.eye(OUT_FEAT)
    s[OUT_FEAT:, :] = np.eye(OUT_FEAT)
    return s.astype(ml_dtypes.bfloat16)


def _block_order(c):
    """Column blocks for core c; None marks the poison block."""
    if c < 4:
        return [c, c + 1, c + 2, c + 3, c + 4]
    return [c, (c + 1) % 8, (c + 2) % 8, (c + 3) % 8, None]


def _in_maps(x, T):
    bf16 = ml_dtypes.bfloat16
    Tb = np.ascontiguousarray(T.reshape(IN_FEAT, OK)).astype(bf16)
    selb = _sel_host()
    sel2b = _sel2_host()
    dmapb = _dmap_host()
    xT = np.ascontiguousarray(x.T)
    maps = []
    for c in range(N_CORES):
        xTc = np.empty((IN_FEAT, FD), dtype=np.float32)
        for pos, b in enumerate(_block_order(c)):
            if b is None:
                xTc[:, 64 * pos:64 * (pos + 1)] = POISON
            else:
                xTc[:, 64 * pos:64 * (pos + 1)] = xT[:, 64 * b:64 * (b + 1)]
        maps.append({"xT": xTc.astype(bf16), "Tm": Tb, "sel": selb,
                     "sel2": sel2b, "dmap": dmapb})
    return maps


def _gather(results):
    """results: per-core dict with rowS2 [128, NPAIR], accS [64, FD] and
    eLast [128, FD] (the last pair's raw exp tile, folded here)."""
    mbd = np.zeros((BATCH, OUT_FEAT), dtype=np.float32)
    for c in range(N_CORES):
        rs = np.array(results[c]["rowS2"], dtype=np.float32)
        eL = np.asarray(results[c]["eLast"], dtype=np.float32)
        rs[:, NPAIR - 1] = eL.sum(axis=1)
        # partitions [64s:64s+64] of column m are the rowsum of i = 2m+s
        rows = rs.reshape(2, OUT_FEAT, NPAIR).transpose(2, 0, 1)
        mbd[64 * c:64 * (c + 1), :] += rows.reshape(ROWB, OUT_FEAT)
        acc = np.asarray(results[c]["accS"], dtype=np.float32)  # [o, j]
        acc = acc + eL[0:OUT_FEAT] + eL[OUT_FEAT:128]
        for pos, b in enumerate(_block_order(c)):
            if pos == 0 or b is None:
                continue  # own diag block is fully in rowsums; poison dropped
            mbd[64 * b:64 * (b + 1), :] += acc[:, 64 * pos:64 * (pos + 1)].T
    mbd -= 1.0
    return mbd


def kernel(x, T):
    from concourse import bass_utils

    x = np.asarray(x, dtype=np.float32)
    T = np.asarray(T, dtype=np.float32)

    if "nc" not in _cache:
        _cache["nc"] = _build_nc()
    nc = _cache["nc"]

    res = bass_utils.run_bass_kernel_spmd(
        nc, _in_maps(x, T), core_ids=list(range(N_CORES)))

    mbd = _gather(res.results)
    return np.concatenate([x, mbd], axis=1)



# revision 3
# speedup vs baseline: 1.3341x; 1.3341x over previous
"""MiniBatchDiscrimination kernel, v6: fp8 DoubleRow reduction for the
even-i half + triangular diagonal block + per-pair variable widths.

Math per core (row block of 64 i's x FD=320 j columns spanning 5 blocks,
diag block first):
  Mt[(o,k), j] = M^T in bf16 (16 partition-tiles of 128 = 4 o x 32 k).
  Pair m covers i=(2m, 2m+1) and columns [2m, 320): the diagonal block is
  computed as an upper triangle; the host mirrors column sums back onto
  the diagonal rows (pos-0 mirror), so each unordered pair is computed
  exactly once.  Couple columns (2m, 2m+1) are excluded from the colsum
  (their pair rides the two rowsums); every rowsum includes its own
  self-term exp(0)=1, subtracted globally on the host.

  EVEN half (i=2m, psd partitions 0:64): 16 elementwise tiles in fp8e4
  (4 DVE relu + 6 ACT |d| + 6 Pool relu), written into 8 paired buffers
  [128, 2*FD]; k-reduction by 8 fp8 DoubleRow matmuls (0.5 cyc/row,
  dst partitions 0:64 as the ISA requires).
  ODD half (i=2m+1, partitions 64:128): 16 bf16 relu tiles on DVE
  (tensor_scalar 4x mode), reduced by 16 plain 64-wide matmuls at
  tile_position (0, 64).

  |d| = 2*relu(d) - d on relu tiles: per-half S sums (S_even over the
  relu tiles of the even half, S_odd over all odd tiles) land via one
  -I dmap matmul per pair (also resets the PSUM bank via start=True);
  +S_i rides the Exp bias.  The bf16 S cancellation makes the diagonal
  exp exactly 1.  One Exp per pair [128, W] with accum_out rowsums;
  one colsum matmul per pair (range [2m+2, 320)) into a persistent
  PSUM bank.

Sharding: symmetric-pair blocks as v4/v5 (cores 4-7 carry one poisoned
block, POISON=0.5 via xT so poisoned |d| stays in fp8 range while
exp(-D) still underflows to 0); host adds row-sums, mirrored column
sums (now including the diagonal block), and subtracts the
self-similarity 1.
"""

import numpy as np
import ml_dtypes
from contextlib import ExitStack

BATCH, IN_FEAT, OUT_FEAT, KERNEL_DIM = 512, 512, 64, 32
N_CORES = 8
ROWB = BATCH // N_CORES          # 64 rows of i per core
NPAIR = ROWB // 2                # 32 exp/psum pairs
OK = OUT_FEAT * KERNEL_DIM       # 2048 flattened (o,k)
NT = OK // 128                   # 16 partition-tiles of (o,k)
NBLK = 5                         # column blocks per core
FD = NBLK * 64                   # 320
POISON = 0.5                     # fp8-safe: |d| stays < 240, D ~ 600

# even-half engine split (fp8 tiles); odd half is all DVE bf16
EVEN_DVE = (0, 1, 14, 15)
EVEN_POOL = (4, 5, 8, 9, 12, 13)
EVEN_ACT = (2, 3, 6, 7, 10, 11)           # |d| via activation Abs
DR_PAIRS = tuple((2 * p, 2 * p + 1) for p in range(8))  # fp8 buffer pairs
WARM = 70                        # PE p-state warm matmuls over the DMA window

DVE_BUFS = 84                    # odd bf16 ad tiles (16/pair, ~5 pairs)
F8_BUFS = 48                     # paired fp8 buffers (8/pair, 6 pairs)
E_BUFS = 8                       # exp output tiles
# prolog pair depth per even-engine (ACT shallow so S_bf lands early)
PRO_E_DVE = 4
PRO_E_POOL = 3
PRO_E_ACT = 2
PRO_ODD = 3                      # prolog pair depth for odd (DVE bf16)
DR_ORDER = (0, 7, 1, 3, 5, 2, 4, 6)   # DVE-, ACT-, POOL-fed

_cache = {}


def _build_nc(split_waits=True):
    import concourse.bass as bass
    import concourse.mybir as mybir
    import concourse.tile as tile

    dt = mybir.dt
    AF = mybir.ActivationFunctionType
    OP = mybir.AluOpType

    nc = bass.Bass("TRN2", target_bir_lowering=False, debug=False,
                   num_devices=N_CORES)

    # selpk packs [selB (1.0 S weights) | sel2x (2.0 stream weights) |
    # sel2 (colsum fold) | dmap2 (-I)] into one bf16 DMA; T and xT land
    # as single chunk-concatenated DMAs to keep the serial HWDGE short
    SELPK_W = 2 * NT * OUT_FEAT + OUT_FEAT + 128
    xT_d = nc.dram_tensor("xT", [IN_FEAT, FD], dt.bfloat16, kind="ExternalInput")
    T_d = nc.dram_tensor("Tm", [IN_FEAT, OK], dt.bfloat16, kind="ExternalInput")
    selpk_d = nc.dram_tensor("selpk", [128, SELPK_W], dt.bfloat16,
                             kind="ExternalInput")
    selDR_d = nc.dram_tensor("selDR", [128, len(DR_PAIRS) * 2 * OUT_FEAT],
                             dt.float8e4, kind="ExternalInput")
    rows_d = nc.dram_tensor("rowS2", [128, NPAIR], dt.float32,
                            kind="ExternalOutput")
    acc_d = nc.dram_tensor("accS", [OUT_FEAT, FD], dt.float32,
                           kind="ExternalOutput")
    # last pair's exp tile goes out raw over its range [62, 320)
    EL_W = FD - 2 * (NPAIR - 1)
    eL_d = nc.dram_tensor("eLast", [128, EL_W], dt.bfloat16,
                          kind="ExternalOutput")

    # tile -> (even engine kind, relu?) ; odd half is all ('dve', relu)
    even_eng = {}
    for t in EVEN_DVE:
        even_eng[t] = 'dve'
    for t in EVEN_POOL:
        even_eng[t] = 'pool'
    for t in EVEN_ACT:
        even_eng[t] = 'act'
    even_relu = [t for t in range(NT) if even_eng[t] != 'act']
    # fp8 buffer slot of tile t: (pair index, half)
    f8_slot = {}
    for pidx, (tl, th) in enumerate(DR_PAIRS):
        f8_slot[tl] = (pidx, 0)
        f8_slot[th] = (pidx, 1)

    with tile.TileContext(nc) as tc, ExitStack() as ctx:
        const = ctx.enter_context(tc.tile_pool(name="const", bufs=1))
        mtp = ctx.enter_context(tc.tile_pool(name="mt", bufs=NT))
        psA = ctx.enter_context(
            tc.tile_pool(name="psA", bufs=2, space=bass.MemorySpace.PSUM))
        psSp = ctx.enter_context(
            tc.tile_pool(name="psS", bufs=1, space=bass.MemorySpace.PSUM))
        psD = ctx.enter_context(
            tc.tile_pool(name="psD", bufs=4, space=bass.MemorySpace.PSUM))
        psC = ctx.enter_context(
            tc.tile_pool(name="psC", bufs=1, space=bass.MemorySpace.PSUM))
        workV = ctx.enter_context(tc.tile_pool(name="workV", bufs=DVE_BUFS))
        workF = ctx.enter_context(tc.tile_pool(name="workF", bufs=F8_BUFS))
        ep = ctx.enter_context(tc.tile_pool(name="e", bufs=E_BUFS))

        # four input DMAs total (HWDGE descriptor-gen is serial, ~630ns
        # each), split between the SP and ACT queues
        xTall = const.tile([128, 4 * FD], dt.bfloat16, tag="xTall")
        nc.scalar.dma_start(
            xTall[:].rearrange("p (kc c) -> p kc c", kc=4),
            xT_d[:].rearrange("(kc p) c -> p kc c", kc=4))
        # T arrives as four column-chunk DMAs (tiles 4q..4q+3 each) so
        # the M setup starts after ~1/4 of the transfer
        Tall = const.tile([128, 4 * OK], dt.bfloat16, tag="Tall")
        t3 = Tall[:].rearrange("p (kc c) -> p kc c", kc=4)
        d3 = T_d[:].rearrange("(kc p) c -> p kc c", kc=4)
        CQ = OK // 4
        for q in range(4):
            eng = nc.sync if q % 2 == 0 else nc.scalar
            eng.dma_start(t3[:, :, q * CQ:(q + 1) * CQ],
                          d3[:, :, q * CQ:(q + 1) * CQ])
        selpk = const.tile([128, SELPK_W], dt.bfloat16, tag="selpk")
        nc.scalar.dma_start(selpk[:], selpk_d[:])
        selDR = const.tile([128, len(DR_PAIRS) * 2 * OUT_FEAT], dt.float8e4,
                           tag="selDR")
        nc.sync.dma_start(selDR[:], selDR_d[:])
        Tsb = [Tall[:, kc * OK:(kc + 1) * OK] for kc in range(4)]
        xTsb = [xTall[:, kc * FD:(kc + 1) * FD] for kc in range(4)]
        selB = selpk[:, 0:NT * OUT_FEAT]
        sel2x = selpk[:, NT * OUT_FEAT:2 * NT * OUT_FEAT]
        sel2 = selpk[:, 2 * NT * OUT_FEAT:2 * NT * OUT_FEAT + OUT_FEAT]
        dmap2 = selpk[:, 2 * NT * OUT_FEAT + OUT_FEAT:SELPK_W]
        # 1x128 zero weight: K=1 start=True matmul resets a PSUM bank
        zeroW = const.tile([1, 128], dt.bfloat16, tag="zeroW")
        nc.vector.memset(zeroW[:], 0.0)
        rowS2a = const.tile([128, NPAIR // 2], dt.float32, tag="rowS2a")
        rowS2b = const.tile([128, NPAIR // 2], dt.float32, tag="rowS2b")
        accS = const.tile([OUT_FEAT, FD], dt.float32, tag="accS")
        # scalar columns (bf16 Mt values recast to fp32 so the diagonal
        # difference is exactly 0), one tile per consuming engine
        mcolV = const.tile([128, NT * ROWB], dt.float32, tag="mcolV")
        mcolA = const.tile([128, len(EVEN_ACT) * ROWB], dt.float32,
                           tag="mcolA")
        mcolP = const.tile([128, len(EVEN_POOL) * ROWB], dt.float32,
                           tag="mcolP")
        aidx = {t: i for i, t in enumerate(EVEN_ACT)}
        pidx_ = {t: i for i, t in enumerate(EVEN_POOL)}

        # warm the PE clock (p-state) during the input DMA window; the
        # warm bank is the future psS bank (its opener resets it anyway)
        psS = psSp.tile([128, 512], dt.float32, tag="psS")
        for w in range(WARM):
            nc.tensor.matmul(psS[:, 0:128], zeroW[:], zeroW[:],
                             start=True, stop=True)

        # M setup: 16 tiles in T-chunk arrival order; S matmuls trail by 2
        nc.tensor.matmul(psS[:, 510:512], zeroW[0:1, 0:128],
                         zeroW[0:1, 0:2], start=True, stop=True)
        # S_even rows of the ACT (|d|) tiles carry no correction: zero the
        # whole even half explicitly (real PSUM powers up with garbage)
        zrow = const.tile([1, FD], dt.bfloat16, tag="zrow")
        nc.vector.memset(zrow[:], 0.0)
        nc.tensor.matmul(psS[0:64, 0:FD], zeroW[0:1, 0:64], zrow[:],
                         start=False, stop=False, skip_group_check=True,
                         tile_position=(0, 0))

        mts = [None] * NT
        order = list(range(NT))

        def s_matmul(t):
            # S_even rows 0:64 for even relu tiles; S_odd rows 64:128 (all)
            w = selB[:, t * OUT_FEAT:(t + 1) * OUT_FEAT]
            if t in even_relu:
                nc.tensor.matmul(psS[0:64, 0:FD], w, mts[t][:],
                                 start=False, stop=False,
                                 skip_group_check=True, tile_position=(0, 0))
            nc.tensor.matmul(psS[64:128, 0:FD], w, mts[t][:],
                             start=False, stop=False,
                             skip_group_check=True, tile_position=(0, 64))

        def absdiff_even(t, m, bufs):
            """fp8 tile for i=2m into its half of the paired buffer."""
            i = 2 * m
            c0 = 2 * m
            pidx, half = f8_slot[t]
            dst = bufs[pidx][:, half * FD + c0: (half + 1) * FD]
            eng = even_eng[t]
            if eng == 'act':
                sc = mcolA[:, aidx[t] * ROWB + i: aidx[t] * ROWB + i + 1]
                nc.scalar.activation(dst, mts[t][:, c0:FD], AF.Abs,
                                     bias=sc, scale=-1.0)
            elif eng == 'pool':
                sc = mcolP[:, pidx_[t] * ROWB + i: pidx_[t] * ROWB + i + 1]
                nc.gpsimd.tensor_scalar(dst, mts[t][:, c0:FD], sc, 0.0,
                                        op0=OP.subtract, op1=OP.max)
            else:
                sc = mcolV[:, t * ROWB + i: t * ROWB + i + 1]
                nc.vector.tensor_scalar(dst, mts[t][:, c0:FD], sc, 0.0,
                                        op0=OP.subtract, op1=OP.max)

        def absdiff_odd(t, m, name):
            i = 2 * m + 1
            c0 = 2 * m
            sc = mcolV[:, t * ROWB + i: t * ROWB + i + 1]
            ad = workV.tile([128, FD], dt.bfloat16, tag="adV", name=name)
            nc.vector.tensor_scalar(ad[:, c0:FD], mts[t][:, c0:FD], sc, 0.0,
                                    op0=OP.subtract, op1=OP.max)
            return ad

        # prolog: the first pairs' elementwise is emitted tile-by-tile as
        # the T chunks land, so no engine idles behind another's inputs
        eng_pro = {'dve': PRO_E_DVE, 'pool': PRO_E_POOL, 'act': PRO_E_ACT}
        PRO_E = {t: eng_pro[even_eng[t]] for t in range(NT)}
        PRO_ODD_T = {t: PRO_ODD for t in range(NT)}
        PRO_MAX = max(PRO_ODD, *eng_pro.values())
        f8bufs_m = {m: [workF.tile([128, 2 * FD], dt.float8e4, tag="adF",
                                   name=f"f8_{m}_{p}")
                        for p in range(len(DR_PAIRS))] for m in range(PRO_MAX)}
        ado_m = {m: {} for m in range(PRO_MAX)}

        def prolog(t):
            for m in range(PRO_E[t]):
                absdiff_even(t, m, f8bufs_m[m])
            for m in range(PRO_ODD_T[t]):
                ado_m[m][t] = absdiff_odd(t, m, f"ad_{m}_{t}")

        for n, t in enumerate(order):
            ps = psA.tile([128, FD], dt.float32)
            for kc in range(4):
                nc.tensor.matmul(ps[:],
                                 Tsb[kc][:, t * 128:(t + 1) * 128],
                                 xTsb[kc],
                                 start=(kc == 0), stop=(kc == 3))
            mt_t = mtp.tile([128, FD], dt.bfloat16, tag="mt")
            if t % 4 == 1:
                nc.scalar.copy(mt_t[:], ps[:])
            else:
                nc.vector.tensor_copy(mt_t[:], ps[:])
            mts[t] = mt_t
            # scalar-column copies on the consuming engines
            nc.vector.tensor_copy(mcolV[:, t * ROWB:(t + 1) * ROWB],
                                  mt_t[:, 0:ROWB])
            if t in EVEN_ACT:
                ci = aidx[t]
                nc.scalar.copy(mcolA[:, ci * ROWB:(ci + 1) * ROWB],
                               mt_t[:, 0:ROWB])
            elif t in EVEN_POOL:
                ci = pidx_[t]
                nc.gpsimd.tensor_copy(mcolP[:, ci * ROWB:(ci + 1) * ROWB],
                                      mt_t[:, 0:ROWB])
            if n >= 1:
                prolog(order[n - 1])
            if n >= 2:
                s_matmul(order[n - 2])
        prolog(order[NT - 1])
        s_matmul(order[NT - 2])
        s_matmul(order[NT - 1])

        # S in bf16 (the dmap matmul and the exp bias both derive from
        # S_bf, so the diagonal cancellation is exact)
        S_bf = const.tile([128, FD], dt.bfloat16, tag="S_bf")
        nc.scalar.activation(S_bf[:], psS[:, 0:FD], AF.Copy)
        # exp bias: Sneg2[p, m] = -S_bf[p, 2m + (p // 64)]
        Sneg2 = const.tile([128, NPAIR], dt.float32, tag="Sneg2")
        nc.scalar.activation(Sneg2[0:64, :], S_bf[0:64, 0:2 * NPAIR:2],
                             AF.Copy, scale=-1.0)
        nc.scalar.activation(Sneg2[64:128, :], S_bf[64:128, 1:2 * NPAIR:2],
                             AF.Copy, scale=-1.0)

        psc = psC.tile([OUT_FEAT, 512], dt.float32, tag="psc")

        def exp_pair(psd, m, accum=True):
            c0 = 2 * m
            e_t = ep.tile([128, FD], dt.bfloat16, tag="e", name=f"e_{m}")
            half, col = divmod(m, NPAIR // 2)
            rs = rowS2b if half else rowS2a
            kw = {"accum_out": rs[:, col:col + 1]} if accum else {}
            nc.scalar.activation(e_t[:, c0:FD], psd[:, c0:FD], AF.Exp,
                                 scale=-1.0, bias=Sneg2[:, m:m + 1], **kw)
            return e_t

        def colsum(e_t, m, stop=False):
            c0 = 2 * m + 2
            nc.tensor.matmul(psc[:, c0:FD], sel2, e_t[:, c0:FD],
                             start=False, stop=stop, skip_group_check=True)

        DEFER = 2                # ramp pairs with deferred dmap/exp
        deferred_dmap = []
        pending_exps = []        # (psd, m) awaiting exp
        done_e = {}              # m -> e tile awaiting colsum

        for m in range(NPAIR):
            last = m == NPAIR - 1
            c0 = 2 * m
            # elementwise: DVE's fp8 tiles first so the first DR matmuls
            # have input early; ACT/Pool fill their halves in parallel
            # (prolog pairs were already emitted tile-by-tile above)
            if m < PRO_MAX:
                f8bufs = f8bufs_m[m]
                ado = ado_m[m]
            else:
                f8bufs = [workF.tile([128, 2 * FD], dt.float8e4, tag="adF",
                                     name=f"f8_{m}_{p}")
                          for p in range(len(DR_PAIRS))]
                ado = {}
            for t in EVEN_DVE + EVEN_POOL + EVEN_ACT:
                if m >= PRO_E[t]:
                    absdiff_even(t, m, f8bufs)
            for t in range(NT):
                if m >= PRO_ODD_T[t]:
                    ado[t] = absdiff_odd(t, m, f"ad_{m}_{t}")
            if last:
                # emit exp(30) before the last pair's matmuls: its colsum
                # closes psc mid-stream so the accS tail overlaps pair 31
                psd_e, m_e = pending_exps.pop(0)
                done_e[m_e] = exp_pair(psd_e, m_e)
            if m == 2:
                # open the psc group: pending-zero the bank and write the
                # two never-covered columns (real PSUM powers up dirty)
                nc.tensor.matmul(psc[:, 0:2], zeroW[0:1, 0:OUT_FEAT],
                                 zeroW[0:1, 0:2], start=True, stop=False,
                                 skip_group_check=True)
            for mm in sorted(k for k in done_e if k <= m - 2):
                colsum(done_e.pop(mm), mm)
            # dmap: resets the bank (start=True pending-zeros the 2KB
            # region) and writes -S_half(p)[o(p), j] to all 128 partitions.
            # For the ramp pairs (< DEFER) the reset comes from a cheap K=1
            # zero matmul instead, so the streams can run tile-by-tile as
            # the T chunks land; their dmaps (which need S_bf) and exps are
            # emitted in a batch once S_bf exists, keeping the in-order
            # engine queues free of S_bf waits during the ramp
            psd = psD.tile([128, 512], dt.float32, tag="psd",
                           name=f"psd_{m}")
            if m < DEFER:
                nc.tensor.matmul(psd[:, c0:FD], zeroW[0:1, 0:128],
                                 zrow[0:1, c0:FD], start=True, stop=True)
                deferred_dmap.append((psd, m))
            else:
                if deferred_dmap:
                    for psd_d, m_d in deferred_dmap:
                        nc.tensor.matmul(psd_d[:, 2 * m_d:FD], dmap2,
                                         S_bf[:, 2 * m_d:FD],
                                         start=False, stop=False,
                                         skip_group_check=True)
                    deferred_dmap = []
                    while len(pending_exps) > 1:
                        psd_e, m_e = pending_exps.pop(0)
                        done_e[m_e] = exp_pair(psd_e, m_e)
                nc.tensor.matmul(psd[:, c0:FD], dmap2, S_bf[:, c0:FD],
                                 start=True, stop=True)
            # even half: 8 fp8 DoubleRow matmuls (dst partitions 0:64)
            # odd half: 16 plain 64-wide bf16 matmuls (tile_position (0,64))
            dr_emitted = 0
            dr_order = DR_ORDER
            odd_order = list(range(NT))
            for gi, t in enumerate(odd_order):
                if gi % 2 == 0 and dr_emitted < len(DR_PAIRS):
                    p = dr_order[dr_emitted]
                    dr_emitted += 1
                    w3 = selDR[:, p * 2 * OUT_FEAT:(p + 1) * 2 * OUT_FEAT] \
                        .rearrange("q (two m_) -> q two m_", two=2)
                    x3 = f8bufs[p][:].rearrange("q (two w_) -> q two w_",
                                                two=2)[:, :, c0:FD]
                    nc.tensor.matmul(
                        psd[0:64, c0:FD], w3, x3,
                        start=False, stop=False, skip_group_check=True,
                        perf_mode=mybir.MatmulPerfMode.DoubleRow,
                        tile_position=(0, 0))
                if last and gi == 10:
                    # close the psc group mid-stream: accS copies + DMA
                    # then overlap the rest of pair 31
                    colsum(done_e[m - 1], m - 1, stop=True)
                nc.tensor.matmul(
                    psd[64:128, c0:FD],
                    sel2x[:, t * OUT_FEAT:(t + 1) * OUT_FEAT],
                    ado[t][:, c0:FD],
                    start=False, stop=False, skip_group_check=True,
                    tile_position=(0, 64))
            if last:
                HF = FD // 2
                nc.vector.tensor_copy(accS[:, 0:HF], psc[:, 0:HF])
                nc.vector.tensor_copy(accS[:, HF:FD], psc[:, HF:FD])
                nc.sync.dma_start(acc_d[:], accS[:])
                nc.sync.dma_start(rows_d[:, NPAIR // 2:NPAIR - 1],
                                  rowS2b[:, 0:NPAIR // 2 - 1])
            # exp of the PREVIOUS pair (its PE wait is already satisfied,
            # so ACT never stalls); its colsum is emitted one pair later
            if m >= DEFER and pending_exps:
                psd_e, m_e = pending_exps.pop(0)
                done_e[m_e] = exp_pair(psd_e, m_e)
                if m_e == NPAIR // 2 - 1:
                    nc.sync.dma_start(rows_d[:, 0:NPAIR // 2], rowS2a[:])
            pending_exps.append((psd, m))
        # tail: exp31's tile ships raw; the host folds its rowsums and
        # colsum contributions
        (psd_e, m_e), = pending_exps
        e_last = exp_pair(psd_e, m_e, accum=False)
        nc.sync.dma_start(eL_d[:], e_last[:, 2 * (NPAIR - 1):FD])

    if split_waits:
        _split_multiwaits(nc, mybir)
    return nc


def _split_multiwaits(nc, mybir):
    """Walrus on this toolchain encodes at most ONE sync-wait command per
    instruction.  Split any instruction with more waits into a chain of
    single-wait Drain carriers on the same engine, inserted immediately
    before it."""
    n = 0
    for fn in nc.m.functions:
        for bb in fn.blocks:
            new_insts = []
            for inst in bb.instructions:
                si = getattr(inst, "sync_info", None)
                if si is not None and si.on_wait and len(si.on_wait) > 1:
                    waits = list(si.on_wait)
                    for w in waits[:-1]:
                        carrier = mybir.InstDrain(
                            name=f"splitw_{n}", engine=inst.engine,
                            ins=[], outs=[],
                            sync_info=mybir.SyncInfo(on_wait=[w],
                                                     on_update=[]))
                        new_insts.append(carrier)
                        n += 1
                    inst.sync_info = mybir.SyncInfo(
                        on_wait=[waits[-1]], on_update=list(si.on_update))
                new_insts.append(inst)
            if n:
                bb.instructions = new_insts


def _even_sets():
    even_eng = {}
    for t in EVEN_DVE:
        even_eng[t] = 'dve'
    for t in EVEN_POOL:
        even_eng[t] = 'pool'
    for t in EVEN_ACT:
        even_eng[t] = 'act'
    even_relu = [t for t in range(NT) if even_eng[t] != 'act']
    return even_eng, even_relu


def _selB_host():
    """1.0 weights: tile t partition (g,k) -> output row 4t+g (mod 64)."""
    s = np.zeros((128, NT * OUT_FEAT), dtype=np.float32)
    for t in range(NT):
        for g in range(4):
            s[32 * g:32 * (g + 1), t * OUT_FEAT + (4 * t + g) % OUT_FEAT] = 1.0
    return s.astype(ml_dtypes.bfloat16)


def _sel2x_host():
    s = np.zeros((128, NT * OUT_FEAT), dtype=np.float32)
    for t in range(NT):
        for g in range(4):
            s[32 * g:32 * (g + 1), t * OUT_FEAT + (4 * t + g) % OUT_FEAT] = 2.0
    return s.astype(ml_dtypes.bfloat16)


def _selDR_host():
    """fp8 DoubleRow weights: pair p = (t_lo, t_hi); ktile 0 -> rows of
    t_lo, ktile 1 -> rows of t_hi; 2.0 on relu tiles, 1.0 on ACT tiles."""
    even_eng, _ = _even_sets()
    s = np.zeros((128, len(DR_PAIRS), 2, OUT_FEAT), dtype=np.float32)
    for p, pair in enumerate(DR_PAIRS):
        for half, t in enumerate(pair):
            v = 1.0 if even_eng[t] == 'act' else 2.0
            for g in range(4):
                s[32 * g:32 * (g + 1), p, half, (4 * t + g) % OUT_FEAT] = v
    return s.reshape(128, len(DR_PAIRS) * 2 * OUT_FEAT) \
        .astype(ml_dtypes.float8_e4m3)


def _dmap2_host():
    return (-np.eye(128, dtype=np.float32)).astype(ml_dtypes.bfloat16)


def _sel2_host():
    s = np.zeros((128, OUT_FEAT), dtype=np.float32)
    s[:OUT_FEAT, :] = np.eye(OUT_FEAT)
    s[OUT_FEAT:, :] = np.eye(OUT_FEAT)
    return s.astype(ml_dtypes.bfloat16)


def _block_order(c):
    """Column blocks for core c; None marks the poison block."""
    if c < 4:
        return [c, c + 1, c + 2, c + 3, c + 4]
    return [c, (c + 1) % 8, (c + 2) % 8, (c + 3) % 8, None]


def _in_maps(x, T):
    bf16 = ml_dtypes.bfloat16
    Tb = np.ascontiguousarray(T.reshape(IN_FEAT, OK)).astype(bf16)
    selpk = np.ascontiguousarray(np.concatenate(
        [_selB_host(), _sel2x_host(), _sel2_host(), _dmap2_host()],
        axis=1))
    selDR = _selDR_host()
    xT = np.ascontiguousarray(x.T)
    maps = []
    for c in range(N_CORES):
        xTc = np.empty((IN_FEAT, FD), dtype=np.float32)
        for pos, b in enumerate(_block_order(c)):
            if b is None:
                xTc[:, 64 * pos:64 * (pos + 1)] = POISON
            else:
                xTc[:, 64 * pos:64 * (pos + 1)] = xT[:, 64 * b:64 * (b + 1)]
        maps.append({"xT": xTc.astype(bf16), "Tm": Tb, "selpk": selpk,
                     "selDR": selDR})
    return maps


def _gather(results):
    """results: per-core dict with rowS2 [128, NPAIR], accS [64, FD] and
    eLast [128, EL_W] (the last pair's raw exp tile, folded here)."""
    EL_C0 = 2 * (NPAIR - 1)      # 62
    mbd = np.zeros((BATCH, OUT_FEAT), dtype=np.float32)
    for c in range(N_CORES):
        rs = np.array(results[c]["rowS2"], dtype=np.float32)
        eL = np.asarray(results[c]["eLast"], dtype=np.float32)  # [128, 258]
        rs[:, NPAIR - 1] = eL.sum(axis=1)
        # partitions [64s:64s+64] of column m are the rowsum of i = 2m+s
        rows = rs.reshape(2, OUT_FEAT, NPAIR).transpose(2, 0, 1)
        mbd[64 * c:64 * (c + 1), :] += rows.reshape(ROWB, OUT_FEAT)
        acc = np.array(results[c]["accS"], dtype=np.float32)  # [o, j]
        # fold the last pair's colsum (cols [64, FD) = eL cols [2:])
        acc[:, EL_C0 + 2:] += eL[0:OUT_FEAT, 2:] + eL[OUT_FEAT:128, 2:]
        for pos, b in enumerate(_block_order(c)):
            if b is None:
                continue
            if pos == 0:
                b = c  # diagonal block: mirror the triangle
            mbd[64 * b:64 * (b + 1), :] += acc[:, 64 * pos:64 * (pos + 1)].T
    mbd -= 1.0  # every rowsum includes its self-term exp(0)=1
    return mbd


def kernel(x, T):
    from concourse import bass_utils

    x = np.asarray(x, dtype=np.float32)
    T = np.asarray(T, dtype=np.float32)

    if "nc" not in _cache:
        _cache["nc"] = _build_nc()
    nc = _cache["nc"]

    res = bass_utils.run_bass_kernel_spmd(
        nc, _in_maps(x, T), core_ids=list(range(N_CORES)))

    mbd = _gather(res.results)
    return np.concatenate([x, mbd], axis=1)


# revision 4
# speedup vs baseline: 1.3580x; 1.0179x over previous
"""MiniBatchDiscrimination kernel, v6: fp8 DoubleRow reduction for the
even-i half + triangular diagonal block + per-pair variable widths.

Math per core (row block of 64 i's x FD=320 j columns spanning 5 blocks,
diag block first):
  Mt[(o,k), j] = M^T in bf16 (16 partition-tiles of 128 = 4 o x 32 k).
  Pair m covers i=(2m, 2m+1) and columns [2m, 320): the diagonal block is
  computed as an upper triangle; the host mirrors column sums back onto
  the diagonal rows (pos-0 mirror), so each unordered pair is computed
  exactly once.  Couple columns (2m, 2m+1) are excluded from the colsum
  (their pair rides the two rowsums); every rowsum includes its own
  self-term exp(0)=1, subtracted globally on the host.

  EVEN half (i=2m, psd partitions 0:64): 16 elementwise tiles in fp8e4
  (4 DVE relu + 6 ACT |d| + 6 Pool relu), written into 8 paired buffers
  [128, 2*FD]; k-reduction by 8 fp8 DoubleRow matmuls (0.5 cyc/row,
  dst partitions 0:64 as the ISA requires).
  ODD half (i=2m+1, partitions 64:128): 16 bf16 relu tiles on DVE
  (tensor_scalar 4x mode), reduced by 16 plain 64-wide matmuls at
  tile_position (0, 64).

  |d| = 2*relu(d) - d on relu tiles: per-half S sums (S_even over the
  relu tiles of the even half, S_odd over all odd tiles) land via one
  -I dmap matmul per pair (also resets the PSUM bank via start=True);
  +S_i rides the Exp bias.  The bf16 S cancellation makes the diagonal
  exp exactly 1.  One Exp per pair [128, W] with accum_out rowsums;
  one colsum matmul per pair (range [2m+2, 320)) into a persistent
  PSUM bank.

Sharding: symmetric-pair blocks as v4/v5 (cores 4-7 carry one poisoned
block, POISON=0.5 via xT so poisoned |d| stays in fp8 range while
exp(-D) still underflows to 0); host adds row-sums, mirrored column
sums (now including the diagonal block), and subtracts the
self-similarity 1.
"""

import numpy as np
import ml_dtypes
from contextlib import ExitStack

BATCH, IN_FEAT, OUT_FEAT, KERNEL_DIM = 512, 512, 64, 32
N_CORES = 8
ROWB = BATCH // N_CORES          # 64 rows of i per core
NPAIR = ROWB // 2                # 32 exp/psum pairs
OK = OUT_FEAT * KERNEL_DIM       # 2048 flattened (o,k)
NT = OK // 128                   # 16 partition-tiles of (o,k)
NBLK = 5                         # column blocks per core
FD = NBLK * 64                   # 320
POISON = 0.5                     # fp8-safe: |d| stays < 240, D ~ 600

# even-half engine split (fp8 tiles); odd half is all DVE bf16
EVEN_DVE = (0, 1, 14, 15)
EVEN_POOL = (4, 5, 8, 9, 12, 13)
EVEN_ACT = (2, 3, 6, 7, 10, 11)           # |d| via activation Abs
DR_PAIRS = tuple((2 * p, 2 * p + 1) for p in range(8))  # fp8 buffer pairs
WARM = 70                        # PE p-state warm matmuls over the DMA window

DVE_BUFS = 84                    # odd bf16 ad tiles (16/pair, ~5 pairs)
F8_BUFS = 48                     # paired fp8 buffers (8/pair, 6 pairs)
E_BUFS = 8                       # exp output tiles
# prolog pair depth per even-engine (ACT shallow so S_bf lands early)
PRO_E_DVE = 4
PRO_E_POOL = 3
PRO_E_ACT = 2
PRO_ODD = 3                      # prolog pair depth for odd (DVE bf16)
DR_ORDER = (0, 7, 1, 3, 5, 2, 4, 6)   # DVE-, ACT-, POOL-fed

_cache = {}


def _build_nc(split_waits=True):
    import concourse.bass as bass
    import concourse.mybir as mybir
    import concourse.tile as tile

    dt = mybir.dt
    AF = mybir.ActivationFunctionType
    OP = mybir.AluOpType

    nc = bass.Bass("TRN2", target_bir_lowering=False, debug=False,
                   num_devices=N_CORES)

    # selpk packs [selB (1.0 S weights) | sel2x (2.0 stream weights) |
    # sel2 (colsum fold) | dmap2 (-I)] into one bf16 DMA; T and xT land
    # as single chunk-concatenated DMAs to keep the serial HWDGE short
    SELPK_W = 2 * NT * OUT_FEAT + OUT_FEAT + 128
    xT_d = nc.dram_tensor("xT", [IN_FEAT, FD], dt.bfloat16, kind="ExternalInput")
    T_d = nc.dram_tensor("Tm", [IN_FEAT, OK], dt.bfloat16, kind="ExternalInput")
    selpk_d = nc.dram_tensor("selpk", [128, SELPK_W], dt.bfloat16,
                             kind="ExternalInput")
    selDR_d = nc.dram_tensor("selDR", [128, len(DR_PAIRS) * 2 * OUT_FEAT],
                             dt.float8e4, kind="ExternalInput")
    rows_d = nc.dram_tensor("rowS2", [128, NPAIR], dt.float32,
                            kind="ExternalOutput")
    acc_d = nc.dram_tensor("accS", [OUT_FEAT, FD], dt.float32,
                           kind="ExternalOutput")
    # last pair's exp tile goes out raw over its range [62, 320)
    EL_W = FD - 2 * (NPAIR - 1)
    eL_d = nc.dram_tensor("eLast", [128, EL_W], dt.bfloat16,
                          kind="ExternalOutput")
    # even pairs' exp tiles ship raw too: their rowsums fold on the host,
    # saving the ACT read-accumulator pass (the DMA path is idle)
    eS_d = nc.dram_tensor("eShip", [128, NPAIR * FD], dt.bfloat16,
                          kind="ExternalOutput")

    # tile -> (even engine kind, relu?) ; odd half is all ('dve', relu)
    even_eng = {}
    for t in EVEN_DVE:
        even_eng[t] = 'dve'
    for t in EVEN_POOL:
        even_eng[t] = 'pool'
    for t in EVEN_ACT:
        even_eng[t] = 'act'
    even_relu = [t for t in range(NT) if even_eng[t] != 'act']
    # fp8 buffer slot of tile t: (pair index, half)
    f8_slot = {}
    for pidx, (tl, th) in enumerate(DR_PAIRS):
        f8_slot[tl] = (pidx, 0)
        f8_slot[th] = (pidx, 1)

    with tile.TileContext(nc) as tc, ExitStack() as ctx:
        const = ctx.enter_context(tc.tile_pool(name="const", bufs=1))
        mtp = ctx.enter_context(tc.tile_pool(name="mt", bufs=NT))
        psA = ctx.enter_context(
            tc.tile_pool(name="psA", bufs=2, space=bass.MemorySpace.PSUM))
        psSp = ctx.enter_context(
            tc.tile_pool(name="psS", bufs=1, space=bass.MemorySpace.PSUM))
        psD = ctx.enter_context(
            tc.tile_pool(name="psD", bufs=4, space=bass.MemorySpace.PSUM))
        psC = ctx.enter_context(
            tc.tile_pool(name="psC", bufs=1, space=bass.MemorySpace.PSUM))
        workV = ctx.enter_context(tc.tile_pool(name="workV", bufs=DVE_BUFS))
        workF = ctx.enter_context(tc.tile_pool(name="workF", bufs=F8_BUFS))
        ep = ctx.enter_context(tc.tile_pool(name="e", bufs=E_BUFS))

        # four input DMAs total (HWDGE descriptor-gen is serial, ~630ns
        # each), split between the SP and ACT queues
        xTall = const.tile([128, 4 * FD], dt.bfloat16, tag="xTall")
        nc.scalar.dma_start(
            xTall[:].rearrange("p (kc c) -> p kc c", kc=4),
            xT_d[:].rearrange("(kc p) c -> p kc c", kc=4))
        # T arrives as four column-chunk DMAs (tiles 4q..4q+3 each) so
        # the M setup starts after ~1/4 of the transfer
        Tall = const.tile([128, 4 * OK], dt.bfloat16, tag="Tall")
        t3 = Tall[:].rearrange("p (kc c) -> p kc c", kc=4)
        d3 = T_d[:].rearrange("(kc p) c -> p kc c", kc=4)
        CQ = OK // 4
        for q in range(4):
            eng = nc.sync if q % 2 == 0 else nc.scalar
            eng.dma_start(t3[:, :, q * CQ:(q + 1) * CQ],
                          d3[:, :, q * CQ:(q + 1) * CQ])
        selpk = const.tile([128, SELPK_W], dt.bfloat16, tag="selpk")
        nc.scalar.dma_start(selpk[:], selpk_d[:])
        selDR = const.tile([128, len(DR_PAIRS) * 2 * OUT_FEAT], dt.float8e4,
                           tag="selDR")
        nc.sync.dma_start(selDR[:], selDR_d[:])
        Tsb = [Tall[:, kc * OK:(kc + 1) * OK] for kc in range(4)]
        xTsb = [xTall[:, kc * FD:(kc + 1) * FD] for kc in range(4)]
        selB = selpk[:, 0:NT * OUT_FEAT]
        sel2x = selpk[:, NT * OUT_FEAT:2 * NT * OUT_FEAT]
        sel2 = selpk[:, 2 * NT * OUT_FEAT:2 * NT * OUT_FEAT + OUT_FEAT]
        dmap2 = selpk[:, 2 * NT * OUT_FEAT + OUT_FEAT:SELPK_W]
        # 1x128 zero weight: K=1 start=True matmul resets a PSUM bank
        zeroW = const.tile([1, 128], dt.bfloat16, tag="zeroW")
        nc.vector.memset(zeroW[:], 0.0)
        rowS2a = const.tile([128, NPAIR // 2], dt.float32, tag="rowS2a")
        rowS2b = const.tile([128, NPAIR // 2], dt.float32, tag="rowS2b")
        # even pairs skip accum_out (host sums their shipped tiles); zero
        # the unused columns so the rowS2 DMA reads defined memory
        nc.vector.memset(rowS2a[:], 0.0)
        nc.vector.memset(rowS2b[:], 0.0)
        accS = const.tile([OUT_FEAT, FD], dt.float32, tag="accS")
        # scalar columns (bf16 Mt values recast to fp32 so the diagonal
        # difference is exactly 0), one tile per consuming engine
        mcolV = const.tile([128, NT * ROWB], dt.float32, tag="mcolV")
        mcolA = const.tile([128, len(EVEN_ACT) * ROWB], dt.float32,
                           tag="mcolA")
        mcolP = const.tile([128, len(EVEN_POOL) * ROWB], dt.float32,
                           tag="mcolP")
        aidx = {t: i for i, t in enumerate(EVEN_ACT)}
        pidx_ = {t: i for i, t in enumerate(EVEN_POOL)}

        # warm the PE clock (p-state) during the input DMA window; the
        # warm bank is the future psS bank (its opener resets it anyway)
        psS = psSp.tile([128, 512], dt.float32, tag="psS")
        for w in range(WARM):
            nc.tensor.matmul(psS[:, 0:128], zeroW[:], zeroW[:],
                             start=True, stop=True)

        # M setup: 16 tiles in T-chunk arrival order; S matmuls trail by 2
        nc.tensor.matmul(psS[:, 510:512], zeroW[0:1, 0:128],
                         zeroW[0:1, 0:2], start=True, stop=True)
        # S_even rows of the ACT (|d|) tiles carry no correction: zero the
        # whole even half explicitly (real PSUM powers up with garbage)
        zrow = const.tile([1, FD], dt.bfloat16, tag="zrow")
        nc.vector.memset(zrow[:], 0.0)
        nc.tensor.matmul(psS[0:64, 0:FD], zeroW[0:1, 0:64], zrow[:],
                         start=False, stop=False, skip_group_check=True,
                         tile_position=(0, 0))

        mts = [None] * NT
        order = list(range(NT))

        def s_matmul(t):
            # S_even rows 0:64 for even relu tiles; S_odd rows 64:128 (all)
            w = selB[:, t * OUT_FEAT:(t + 1) * OUT_FEAT]
            if t in even_relu:
                nc.tensor.matmul(psS[0:64, 0:FD], w, mts[t][:],
                                 start=False, stop=False,
                                 skip_group_check=True, tile_position=(0, 0))
            nc.tensor.matmul(psS[64:128, 0:FD], w, mts[t][:],
                             start=False, stop=False,
                             skip_group_check=True, tile_position=(0, 64))

        def absdiff_even(t, m, bufs):
            """fp8 tile for i=2m into its half of the paired buffer."""
            i = 2 * m
            c0 = 2 * m
            pidx, half = f8_slot[t]
            dst = bufs[pidx][:, half * FD + c0: (half + 1) * FD]
            eng = even_eng[t]
            if eng == 'act':
                sc = mcolA[:, aidx[t] * ROWB + i: aidx[t] * ROWB + i + 1]
                nc.scalar.activation(dst, mts[t][:, c0:FD], AF.Abs,
                                     bias=sc, scale=-1.0)
            elif eng == 'pool':
                sc = mcolP[:, pidx_[t] * ROWB + i: pidx_[t] * ROWB + i + 1]
                nc.gpsimd.tensor_scalar(dst, mts[t][:, c0:FD], sc, 0.0,
                                        op0=OP.subtract, op1=OP.max)
            else:
                sc = mcolV[:, t * ROWB + i: t * ROWB + i + 1]
                nc.vector.tensor_scalar(dst, mts[t][:, c0:FD], sc, 0.0,
                                        op0=OP.subtract, op1=OP.max)

        def absdiff_odd(t, m, name):
            i = 2 * m + 1
            c0 = 2 * m
            sc = mcolV[:, t * ROWB + i: t * ROWB + i + 1]
            ad = workV.tile([128, FD], dt.bfloat16, tag="adV", name=name)
            nc.vector.tensor_scalar(ad[:, c0:FD], mts[t][:, c0:FD], sc, 0.0,
                                    op0=OP.subtract, op1=OP.max)
            return ad

        # prolog: the first pairs' elementwise is emitted tile-by-tile as
        # the T chunks land, so no engine idles behind another's inputs
        eng_pro = {'dve': PRO_E_DVE, 'pool': PRO_E_POOL, 'act': PRO_E_ACT}
        PRO_E = {t: eng_pro[even_eng[t]] for t in range(NT)}
        PRO_ODD_T = {t: PRO_ODD for t in range(NT)}
        PRO_MAX = max(PRO_ODD, *eng_pro.values())
        f8bufs_m = {m: [workF.tile([128, 2 * FD], dt.float8e4, tag="adF",
                                   name=f"f8_{m}_{p}")
                        for p in range(len(DR_PAIRS))] for m in range(PRO_MAX)}
        ado_m = {m: {} for m in range(PRO_MAX)}

        def prolog(t):
            for m in range(PRO_E[t]):
                absdiff_even(t, m, f8bufs_m[m])
            for m in range(PRO_ODD_T[t]):
                ado_m[m][t] = absdiff_odd(t, m, f"ad_{m}_{t}")

        for n, t in enumerate(order):
            ps = psA.tile([128, FD], dt.float32)
            for kc in range(4):
                nc.tensor.matmul(ps[:],
                                 Tsb[kc][:, t * 128:(t + 1) * 128],
                                 xTsb[kc],
                                 start=(kc == 0), stop=(kc == 3))
            mt_t = mtp.tile([128, FD], dt.bfloat16, tag="mt")
            if t % 4 == 1:
                nc.scalar.copy(mt_t[:], ps[:])
            else:
                nc.vector.tensor_copy(mt_t[:], ps[:])
            mts[t] = mt_t
            # scalar-column copies on the consuming engines
            nc.vector.tensor_copy(mcolV[:, t * ROWB:(t + 1) * ROWB],
                                  mt_t[:, 0:ROWB])
            if t in EVEN_ACT:
                ci = aidx[t]
                nc.scalar.copy(mcolA[:, ci * ROWB:(ci + 1) * ROWB],
                               mt_t[:, 0:ROWB])
            elif t in EVEN_POOL:
                ci = pidx_[t]
                nc.gpsimd.tensor_copy(mcolP[:, ci * ROWB:(ci + 1) * ROWB],
                                      mt_t[:, 0:ROWB])
            if n >= 1:
                prolog(order[n - 1])
            if n >= 2:
                s_matmul(order[n - 2])
        prolog(order[NT - 1])
        s_matmul(order[NT - 2])
        s_matmul(order[NT - 1])

        # S in bf16 (the dmap matmul and the exp bias both derive from
        # S_bf, so the diagonal cancellation is exact)
        S_bf = const.tile([128, FD], dt.bfloat16, tag="S_bf")
        nc.scalar.activation(S_bf[:], psS[:, 0:FD], AF.Copy)
        # exp bias: Sneg2[p, m] = -S_bf[p, 2m + (p // 64)]
        Sneg2 = const.tile([128, NPAIR], dt.float32, tag="Sneg2")
        nc.scalar.activation(Sneg2[0:64, :], S_bf[0:64, 0:2 * NPAIR:2],
                             AF.Copy, scale=-1.0)
        nc.scalar.activation(Sneg2[64:128, :], S_bf[64:128, 1:2 * NPAIR:2],
                             AF.Copy, scale=-1.0)

        psc = psC.tile([OUT_FEAT, 512], dt.float32, tag="psc")

        def exp_pair(psd, m, accum=True):
            c0 = 2 * m
            e_t = ep.tile([128, FD], dt.bfloat16, tag="e", name=f"e_{m}")
            half, col = divmod(m, NPAIR // 2)
            rs = rowS2b if half else rowS2a
            accum = accum and m % 2 == 1
            kw = {"accum_out": rs[:, col:col + 1]} if accum else {}
            nc.scalar.activation(e_t[:, c0:FD], psd[:, c0:FD], AF.Exp,
                                 scale=-1.0, bias=Sneg2[:, m:m + 1], **kw)
            if not accum and m < NPAIR - 1:
                nc.sync.dma_start(eS_d[:, m * FD + c0:(m + 1) * FD],
                                  e_t[:, c0:FD])
            return e_t

        def colsum(e_t, m, stop=False):
            c0 = 2 * m + 2
            nc.tensor.matmul(psc[:, c0:FD], sel2, e_t[:, c0:FD],
                             start=False, stop=stop, skip_group_check=True)

        DEFER = 2                # ramp pairs with deferred dmap/exp
        deferred_dmap = []
        pending_exps = []        # (psd, m) awaiting exp
        done_e = {}              # m -> e tile awaiting colsum

        for m in range(NPAIR):
            last = m == NPAIR - 1
            c0 = 2 * m
            # elementwise: DVE's fp8 tiles first so the first DR matmuls
            # have input early; ACT/Pool fill their halves in parallel
            # (prolog pairs were already emitted tile-by-tile above)
            if m < PRO_MAX:
                f8bufs = f8bufs_m[m]
                ado = ado_m[m]
            else:
                f8bufs = [workF.tile([128, 2 * FD], dt.float8e4, tag="adF",
                                     name=f"f8_{m}_{p}")
                          for p in range(len(DR_PAIRS))]
                ado = {}
            for t in EVEN_DVE + EVEN_POOL + EVEN_ACT:
                if m >= PRO_E[t]:
                    absdiff_even(t, m, f8bufs)
            for t in range(NT):
                if m >= PRO_ODD_T[t]:
                    ado[t] = absdiff_odd(t, m, f"ad_{m}_{t}")
            if last:
                # emit exp(30) before the last pair's matmuls: its colsum
                # closes psc mid-stream so the accS tail overlaps pair 31
                psd_e, m_e = pending_exps.pop(0)
                done_e[m_e] = exp_pair(psd_e, m_e)
            if m == 2:
                # open the psc group: pending-zero the bank and write the
                # two never-covered columns (real PSUM powers up dirty)
                nc.tensor.matmul(psc[:, 0:2], zeroW[0:1, 0:OUT_FEAT],
                                 zeroW[0:1, 0:2], start=True, stop=False,
                                 skip_group_check=True)
            for mm in sorted(k for k in done_e if k <= m - 2):
                colsum(done_e.pop(mm), mm)
            # dmap: resets the bank (start=True pending-zeros the 2KB
            # region) and writes -S_half(p)[o(p), j] to all 128 partitions.
            # For the ramp pairs (< DEFER) the reset comes from a cheap K=1
            # zero matmul instead, so the streams can run tile-by-tile as
            # the T chunks land; their dmaps (which need S_bf) and exps are
            # emitted in a batch once S_bf exists, keeping the in-order
            # engine queues free of S_bf waits during the ramp
            psd = psD.tile([128, 512], dt.float32, tag="psd",
                           name=f"psd_{m}")
            if m < DEFER:
                nc.tensor.matmul(psd[:, c0:FD], zeroW[0:1, 0:128],
                                 zrow[0:1, c0:FD], start=True, stop=True)
                deferred_dmap.append((psd, m))
            else:
                if deferred_dmap:
                    for psd_d, m_d in deferred_dmap:
                        nc.tensor.matmul(psd_d[:, 2 * m_d:FD], dmap2,
                                         S_bf[:, 2 * m_d:FD],
                                         start=False, stop=False,
                                         skip_group_check=True)
                    deferred_dmap = []
                    while len(pending_exps) > 1:
                        psd_e, m_e = pending_exps.pop(0)
                        done_e[m_e] = exp_pair(psd_e, m_e)
                nc.tensor.matmul(psd[:, c0:FD], dmap2, S_bf[:, c0:FD],
                                 start=True, stop=True)
            # even half: 8 fp8 DoubleRow matmuls (dst partitions 0:64)
            # odd half: 16 plain 64-wide bf16 matmuls (tile_position (0,64))
            dr_emitted = 0
            dr_order = DR_ORDER
            odd_order = list(range(NT))
            for gi, t in enumerate(odd_order):
                if gi % 2 == 0 and dr_emitted < len(DR_PAIRS):
                    p = dr_order[dr_emitted]
                    dr_emitted += 1
                    w3 = selDR[:, p * 2 * OUT_FEAT:(p + 1) * 2 * OUT_FEAT] \
                        .rearrange("q (two m_) -> q two m_", two=2)
                    x3 = f8bufs[p][:].rearrange("q (two w_) -> q two w_",
                                                two=2)[:, :, c0:FD]
                    nc.tensor.matmul(
                        psd[0:64, c0:FD], w3, x3,
                        start=False, stop=False, skip_group_check=True,
                        perf_mode=mybir.MatmulPerfMode.DoubleRow,
                        tile_position=(0, 0))
                if last and gi == 10:
                    # close the psc group mid-stream: accS copies + DMA
                    # then overlap the rest of pair 31
                    colsum(done_e[m - 1], m - 1, stop=True)
                nc.tensor.matmul(
                    psd[64:128, c0:FD],
                    sel2x[:, t * OUT_FEAT:(t + 1) * OUT_FEAT],
                    ado[t][:, c0:FD],
                    start=False, stop=False, skip_group_check=True,
                    tile_position=(0, 64))
            if last:
                HF = FD // 2
                nc.vector.tensor_copy(accS[:, 0:HF], psc[:, 0:HF])
                nc.vector.tensor_copy(accS[:, HF:FD], psc[:, HF:FD])
                nc.sync.dma_start(acc_d[:], accS[:])
                nc.sync.dma_start(rows_d[:, NPAIR // 2:NPAIR - 1],
                                  rowS2b[:, 0:NPAIR // 2 - 1])
            # exp of the PREVIOUS pair (its PE wait is already satisfied,
            # so ACT never stalls); its colsum is emitted one pair later
            if m >= DEFER and pending_exps:
                psd_e, m_e = pending_exps.pop(0)
                done_e[m_e] = exp_pair(psd_e, m_e)
                if m_e == NPAIR // 2 - 1:
                    nc.sync.dma_start(rows_d[:, 0:NPAIR // 2], rowS2a[:])
            pending_exps.append((psd, m))
        # tail: exp31's tile ships raw; the host folds its rowsums and
        # colsum contributions
        (psd_e, m_e), = pending_exps
        e_last = exp_pair(psd_e, m_e, accum=False)
        nc.sync.dma_start(eL_d[:], e_last[:, 2 * (NPAIR - 1):FD])

    if split_waits:
        _split_multiwaits(nc, mybir)
    return nc


def _split_multiwaits(nc, mybir):
    """Walrus on this toolchain encodes at most ONE sync-wait command per
    instruction.  Split any instruction with more waits into a chain of
    single-wait Drain carriers on the same engine, inserted immediately
    before it."""
    n = 0
    for fn in nc.m.functions:
        for bb in fn.blocks:
            new_insts = []
            for inst in bb.instructions:
                si = getattr(inst, "sync_info", None)
                if si is not None and si.on_wait and len(si.on_wait) > 1:
                    waits = list(si.on_wait)
                    for w in waits[:-1]:
                        carrier = mybir.InstDrain(
                            name=f"splitw_{n}", engine=inst.engine,
                            ins=[], outs=[],
                            sync_info=mybir.SyncInfo(on_wait=[w],
                                                     on_update=[]))
                        new_insts.append(carrier)
                        n += 1
                    inst.sync_info = mybir.SyncInfo(
                        on_wait=[waits[-1]], on_update=list(si.on_update))
                new_insts.append(inst)
            if n:
                bb.instructions = new_insts


def _even_sets():
    even_eng = {}
    for t in EVEN_DVE:
        even_eng[t] = 'dve'
    for t in EVEN_POOL:
        even_eng[t] = 'pool'
    for t in EVEN_ACT:
        even_eng[t] = 'act'
    even_relu = [t for t in range(NT) if even_eng[t] != 'act']
    return even_eng, even_relu


def _selB_host():
    """1.0 weights: tile t partition (g,k) -> output row 4t+g (mod 64)."""
    s = np.zeros((128, NT * OUT_FEAT), dtype=np.float32)
    for t in range(NT):
        for g in range(4):
            s[32 * g:32 * (g + 1), t * OUT_FEAT + (4 * t + g) % OUT_FEAT] = 1.0
    return s.astype(ml_dtypes.bfloat16)


def _sel2x_host():
    s = np.zeros((128, NT * OUT_FEAT), dtype=np.float32)
    for t in range(NT):
        for g in range(4):
            s[32 * g:32 * (g + 1), t * OUT_FEAT + (4 * t + g) % OUT_FEAT] = 2.0
    return s.astype(ml_dtypes.bfloat16)


def _selDR_host():
    """fp8 DoubleRow weights: pair p = (t_lo, t_hi); ktile 0 -> rows of
    t_lo, ktile 1 -> rows of t_hi; 2.0 on relu tiles, 1.0 on ACT tiles."""
    even_eng, _ = _even_sets()
    s = np.zeros((128, len(DR_PAIRS), 2, OUT_FEAT), dtype=np.float32)
    for p, pair in enumerate(DR_PAIRS):
        for half, t in enumerate(pair):
            v = 1.0 if even_eng[t] == 'act' else 2.0
            for g in range(4):
                s[32 * g:32 * (g + 1), p, half, (4 * t + g) % OUT_FEAT] = v
    return s.reshape(128, len(DR_PAIRS) * 2 * OUT_FEAT) \
        .astype(ml_dtypes.float8_e4m3)


def _dmap2_host():
    return (-np.eye(128, dtype=np.float32)).astype(ml_dtypes.bfloat16)


def _sel2_host():
    s = np.zeros((128, OUT_FEAT), dtype=np.float32)
    s[:OUT_FEAT, :] = np.eye(OUT_FEAT)
    s[OUT_FEAT:, :] = np.eye(OUT_FEAT)
    return s.astype(ml_dtypes.bfloat16)


def _block_order(c):
    """Column blocks for core c; None marks the poison block."""
    if c < 4:
        return [c, c + 1, c + 2, c + 3, c + 4]
    return [c, (c + 1) % 8, (c + 2) % 8, (c + 3) % 8, None]


def _in_maps(x, T):
    bf16 = ml_dtypes.bfloat16
    Tb = np.ascontiguousarray(T.reshape(IN_FEAT, OK)).astype(bf16)
    selpk = np.ascontiguousarray(np.concatenate(
        [_selB_host(), _sel2x_host(), _sel2_host(), _dmap2_host()],
        axis=1))
    selDR = _selDR_host()
    xT = np.ascontiguousarray(x.T)
    maps = []
    for c in range(N_CORES):
        xTc = np.empty((IN_FEAT, FD), dtype=np.float32)
        for pos, b in enumerate(_block_order(c)):
            if b is None:
                xTc[:, 64 * pos:64 * (pos + 1)] = POISON
            else:
                xTc[:, 64 * pos:64 * (pos + 1)] = xT[:, 64 * b:64 * (b + 1)]
        maps.append({"xT": xTc.astype(bf16), "Tm": Tb, "selpk": selpk,
                     "selDR": selDR})
    return maps


def _gather(results):
    """results: per-core dict with rowS2 [128, NPAIR], accS [64, FD],
    eLast [128, EL_W] and eShip [128, NPAIR*FD] (raw exp tiles of the
    last + even pairs; rowsums folded here)."""
    EL_C0 = 2 * (NPAIR - 1)      # 62
    mbd = np.zeros((BATCH, OUT_FEAT), dtype=np.float32)
    for c in range(N_CORES):
        rs = np.array(results[c]["rowS2"], dtype=np.float32)
        eL = np.asarray(results[c]["eLast"], dtype=np.float32)  # [128, 258]
        eS = np.asarray(results[c]["eShip"], dtype=np.float32)
        for m in range(0, NPAIR - 1, 2):
            rs[:, m] = eS[:, m * FD + 2 * m:(m + 1) * FD].sum(axis=1)
        rs[:, NPAIR - 1] = eL.sum(axis=1)
        # partitions [64s:64s+64] of column m are the rowsum of i = 2m+s
        rows = rs.reshape(2, OUT_FEAT, NPAIR).transpose(2, 0, 1)
        mbd[64 * c:64 * (c + 1), :] += rows.reshape(ROWB, OUT_FEAT)
        acc = np.array(results[c]["accS"], dtype=np.float32)  # [o, j]
        # fold the last pair's colsum (cols [64, FD) = eL cols [2:])
        acc[:, EL_C0 + 2:] += eL[0:OUT_FEAT, 2:] + eL[OUT_FEAT:128, 2:]
        for pos, b in enumerate(_block_order(c)):
            if b is None:
                continue
            if pos == 0:
                b = c  # diagonal block: mirror the triangle
            mbd[64 * b:64 * (b + 1), :] += acc[:, 64 * pos:64 * (pos + 1)].T
    mbd -= 1.0  # every rowsum includes its self-term exp(0)=1
    return mbd


def kernel(x, T):
    from concourse import bass_utils

    x = np.asarray(x, dtype=np.float32)
    T = np.asarray(T, dtype=np.float32)

    if "nc" not in _cache:
        _cache["nc"] = _build_nc()
    nc = _cache["nc"]

    res = bass_utils.run_bass_kernel_spmd(
        nc, _in_maps(x, T), core_ids=list(range(N_CORES)))

    mbd = _gather(res.results)
    return np.concatenate([x, mbd], axis=1)


# revision 5
# speedup vs baseline: 1.3771x; 1.0140x over previous
"""MiniBatchDiscrimination kernel, v6: fp8 DoubleRow reduction for the
even-i half + triangular diagonal block + per-pair variable widths.

Math per core (row block of 64 i's x FD=320 j columns spanning 5 blocks,
diag block first):
  Mt[(o,k), j] = M^T in bf16 (16 partition-tiles of 128 = 4 o x 32 k).
  Pair m covers i=(2m, 2m+1) and columns [2m, 320): the diagonal block is
  computed as an upper triangle; the host mirrors column sums back onto
  the diagonal rows (pos-0 mirror), so each unordered pair is computed
  exactly once.  Couple columns (2m, 2m+1) are excluded from the colsum
  (their pair rides the two rowsums); every rowsum includes its own
  self-term exp(0)=1, subtracted globally on the host.

  EVEN half (i=2m, psd partitions 0:64): 16 elementwise tiles in fp8e4
  (4 DVE relu + 6 ACT |d| + 6 Pool relu), written into 8 paired buffers
  [128, 2*FD]; k-reduction by 8 fp8 DoubleRow matmuls (0.5 cyc/row,
  dst partitions 0:64 as the ISA requires).
  ODD half (i=2m+1, partitions 64:128): 16 bf16 relu tiles on DVE
  (tensor_scalar 4x mode), reduced by 16 plain 64-wide matmuls at
  tile_position (0, 64).

  |d| = 2*relu(d) - d on relu tiles: per-half S sums (S_even over the
  relu tiles of the even half, S_odd over all odd tiles) land via one
  -I dmap matmul per pair (also resets the PSUM bank via start=True);
  +S_i rides the Exp bias.  The bf16 S cancellation makes the diagonal
  exp exactly 1.  One Exp per pair [128, W] with accum_out rowsums;
  one colsum matmul per pair (range [2m+2, 320)) into a persistent
  PSUM bank.

Sharding: symmetric-pair blocks as v4/v5 (cores 4-7 carry one poisoned
block, POISON=0.5 via xT so poisoned |d| stays in fp8 range while
exp(-D) still underflows to 0); host adds row-sums, mirrored column
sums (now including the diagonal block), and subtracts the
self-similarity 1.
"""

import numpy as np
import ml_dtypes
from contextlib import ExitStack

BATCH, IN_FEAT, OUT_FEAT, KERNEL_DIM = 512, 512, 64, 32
N_CORES = 8
ROWB = BATCH // N_CORES          # 64 rows of i per core
NPAIR = ROWB // 2                # 32 exp/psum pairs
OK = OUT_FEAT * KERNEL_DIM       # 2048 flattened (o,k)
NT = OK // 128                   # 16 partition-tiles of (o,k)
NBLK = 5                         # column blocks per core
FD = NBLK * 64                   # 320
POISON = 0.5                     # fp8-safe: |d| stays < 240, D ~ 600

# even-half engine split (fp8 tiles); odd half is all DVE bf16
EVEN_DVE = (0, 1, 14, 15)
EVEN_POOL = (4, 5, 8, 9, 12, 13)
EVEN_ACT = (2, 3, 6, 7, 10, 11)           # |d| via activation Abs
DR_PAIRS = tuple((2 * p, 2 * p + 1) for p in range(8))  # fp8 buffer pairs
WARM = 70                        # PE p-state warm matmuls over the DMA window

DVE_BUFS = 84                    # odd bf16 ad tiles (16/pair, ~5 pairs)
F8_BUFS = 48                     # paired fp8 buffers (8/pair, 6 pairs)
E_BUFS = 8                       # exp output tiles
# prolog pair depth per even-engine (ACT shallow so S_bf lands early)
PRO_E_DVE = 4
PRO_E_POOL = 3
PRO_E_ACT = 2
PRO_ODD = 3                      # prolog pair depth for odd (DVE bf16)
DR_ORDER = (0, 7, 1, 3, 5, 2, 4, 6)   # DVE-, ACT-, POOL-fed

_cache = {}


def _build_nc(split_waits=True):
    import concourse.bass as bass
    import concourse.mybir as mybir
    import concourse.tile as tile

    dt = mybir.dt
    AF = mybir.ActivationFunctionType
    OP = mybir.AluOpType

    nc = bass.Bass("TRN2", target_bir_lowering=False, debug=False,
                   num_devices=N_CORES)

    # selpk packs [selB (1.0 S weights) | sel2x (2.0 stream weights) |
    # sel2 (colsum fold) | dmap2 (-I)] into one bf16 DMA; T and xT land
    # as single chunk-concatenated DMAs to keep the serial HWDGE short
    SELPK_W = 2 * NT * OUT_FEAT + OUT_FEAT + 128
    xT_d = nc.dram_tensor("xT", [IN_FEAT, FD], dt.bfloat16, kind="ExternalInput")
    T_d = nc.dram_tensor("Tm", [IN_FEAT, OK], dt.bfloat16, kind="ExternalInput")
    selpk_d = nc.dram_tensor("selpk", [128, SELPK_W], dt.bfloat16,
                             kind="ExternalInput")
    selDR_d = nc.dram_tensor("selDR", [128, len(DR_PAIRS) * 2 * OUT_FEAT],
                             dt.float8e4, kind="ExternalInput")
    rows_d = nc.dram_tensor("rowS2", [128, NPAIR], dt.float32,
                            kind="ExternalOutput")
    acc_d = nc.dram_tensor("accS", [OUT_FEAT, FD], dt.float32,
                           kind="ExternalOutput")
    # last pair's exp tile goes out raw over its range [62, 320)
    EL_W = FD - 2 * (NPAIR - 1)
    eL_d = nc.dram_tensor("eLast", [128, EL_W], dt.bfloat16,
                          kind="ExternalOutput")
    # even pairs' exp tiles ship raw too: their rowsums fold on the host,
    # saving the ACT read-accumulator pass (the DMA path is idle)
    eS_d = nc.dram_tensor("eShip", [128, NPAIR * FD], dt.bfloat16,
                          kind="ExternalOutput")

    # tile -> (even engine kind, relu?) ; odd half is all ('dve', relu)
    even_eng = {}
    for t in EVEN_DVE:
        even_eng[t] = 'dve'
    for t in EVEN_POOL:
        even_eng[t] = 'pool'
    for t in EVEN_ACT:
        even_eng[t] = 'act'
    even_relu = [t for t in range(NT) if even_eng[t] != 'act']
    # fp8 buffer slot of tile t: (pair index, half)
    f8_slot = {}
    for pidx, (tl, th) in enumerate(DR_PAIRS):
        f8_slot[tl] = (pidx, 0)
        f8_slot[th] = (pidx, 1)

    with tile.TileContext(nc) as tc, ExitStack() as ctx:
        const = ctx.enter_context(tc.tile_pool(name="const", bufs=1))
        mtp = ctx.enter_context(tc.tile_pool(name="mt", bufs=NT))
        psA = ctx.enter_context(
            tc.tile_pool(name="psA", bufs=2, space=bass.MemorySpace.PSUM))
        psSp = ctx.enter_context(
            tc.tile_pool(name="psS", bufs=1, space=bass.MemorySpace.PSUM))
        psD = ctx.enter_context(
            tc.tile_pool(name="psD", bufs=4, space=bass.MemorySpace.PSUM))
        psC = ctx.enter_context(
            tc.tile_pool(name="psC", bufs=1, space=bass.MemorySpace.PSUM))
        workV = ctx.enter_context(tc.tile_pool(name="workV", bufs=DVE_BUFS))
        workF = ctx.enter_context(tc.tile_pool(name="workF", bufs=F8_BUFS))
        ep = ctx.enter_context(tc.tile_pool(name="e", bufs=E_BUFS))

        # four input DMAs total (HWDGE descriptor-gen is serial, ~630ns
        # each), split between the SP and ACT queues
        xTall = const.tile([128, 4 * FD], dt.bfloat16, tag="xTall")
        nc.scalar.dma_start(
            xTall[:].rearrange("p (kc c) -> p kc c", kc=4),
            xT_d[:].rearrange("(kc p) c -> p kc c", kc=4))
        # T arrives as four column-chunk DMAs (tiles 4q..4q+3 each) so
        # the M setup starts after ~1/4 of the transfer
        Tall = const.tile([128, 4 * OK], dt.bfloat16, tag="Tall")
        t3 = Tall[:].rearrange("p (kc c) -> p kc c", kc=4)
        d3 = T_d[:].rearrange("(kc p) c -> p kc c", kc=4)
        CQ = OK // 4
        for q in range(4):
            eng = nc.sync if q % 2 == 0 else nc.scalar
            eng.dma_start(t3[:, :, q * CQ:(q + 1) * CQ],
                          d3[:, :, q * CQ:(q + 1) * CQ])
        selpk = const.tile([128, SELPK_W], dt.bfloat16, tag="selpk")
        nc.scalar.dma_start(selpk[:], selpk_d[:])
        selDR = const.tile([128, len(DR_PAIRS) * 2 * OUT_FEAT], dt.float8e4,
                           tag="selDR")
        nc.sync.dma_start(selDR[:], selDR_d[:])
        Tsb = [Tall[:, kc * OK:(kc + 1) * OK] for kc in range(4)]
        xTsb = [xTall[:, kc * FD:(kc + 1) * FD] for kc in range(4)]
        selB = selpk[:, 0:NT * OUT_FEAT]
        sel2x = selpk[:, NT * OUT_FEAT:2 * NT * OUT_FEAT]
        sel2 = selpk[:, 2 * NT * OUT_FEAT:2 * NT * OUT_FEAT + OUT_FEAT]
        dmap2 = selpk[:, 2 * NT * OUT_FEAT + OUT_FEAT:SELPK_W]
        # 1x128 zero weight: K=1 start=True matmul resets a PSUM bank
        zeroW = const.tile([1, 128], dt.bfloat16, tag="zeroW")
        nc.vector.memset(zeroW[:], 0.0)
        rowS2a = const.tile([128, NPAIR // 2], dt.float32, tag="rowS2a")
        rowS2b = const.tile([128, NPAIR // 2], dt.float32, tag="rowS2b")
        # even pairs skip accum_out (host sums their shipped tiles); zero
        # the unused columns so the rowS2 DMA reads defined memory
        nc.vector.memset(rowS2a[:], 0.0)
        nc.vector.memset(rowS2b[:], 0.0)
        accS = const.tile([OUT_FEAT, FD], dt.float32, tag="accS")
        # scalar columns (bf16 Mt values recast to fp32 so the diagonal
        # difference is exactly 0), one tile per consuming engine
        mcolV = const.tile([128, NT * ROWB], dt.float32, tag="mcolV")
        mcolA = const.tile([128, len(EVEN_ACT) * ROWB], dt.float32,
                           tag="mcolA")
        mcolP = const.tile([128, len(EVEN_POOL) * ROWB], dt.float32,
                           tag="mcolP")
        aidx = {t: i for i, t in enumerate(EVEN_ACT)}
        pidx_ = {t: i for i, t in enumerate(EVEN_POOL)}

        # warm the PE clock (p-state) during the input DMA window; the
        # warm bank is the future psS bank (its opener resets it anyway)
        psS = psSp.tile([128, 512], dt.float32, tag="psS")
        for w in range(WARM):
            nc.tensor.matmul(psS[:, 0:128], zeroW[:], zeroW[:],
                             start=True, stop=True)

        # M setup: 16 tiles in T-chunk arrival order; S matmuls trail by 2
        nc.tensor.matmul(psS[:, 510:512], zeroW[0:1, 0:128],
                         zeroW[0:1, 0:2], start=True, stop=True)
        # S_even rows of the ACT (|d|) tiles carry no correction: zero the
        # whole even half explicitly (real PSUM powers up with garbage)
        zrow = const.tile([1, FD], dt.bfloat16, tag="zrow")
        nc.vector.memset(zrow[:], 0.0)
        nc.tensor.matmul(psS[0:64, 0:FD], zeroW[0:1, 0:64], zrow[:],
                         start=False, stop=False, skip_group_check=True,
                         tile_position=(0, 0))

        mts = [None] * NT
        order = list(range(NT))

        def s_matmul(t):
            # S_even rows 0:64 for even relu tiles; S_odd rows 64:128 (all)
            w = selB[:, t * OUT_FEAT:(t + 1) * OUT_FEAT]
            if t in even_relu:
                nc.tensor.matmul(psS[0:64, 0:FD], w, mts[t][:],
                                 start=False, stop=False,
                                 skip_group_check=True, tile_position=(0, 0))
            nc.tensor.matmul(psS[64:128, 0:FD], w, mts[t][:],
                             start=False, stop=False,
                             skip_group_check=True, tile_position=(0, 64))

        def absdiff_even(t, m, bufs):
            """fp8 tile for i=2m into its half of the paired buffer."""
            i = 2 * m
            c0 = 2 * m
            pidx, half = f8_slot[t]
            dst = bufs[pidx][:, half * FD + c0: (half + 1) * FD]
            eng = even_eng[t]
            if eng == 'act':
                sc = mcolA[:, aidx[t] * ROWB + i: aidx[t] * ROWB + i + 1]
                nc.scalar.activation(dst, mts[t][:, c0:FD], AF.Abs,
                                     bias=sc, scale=-1.0)
            elif eng == 'pool':
                sc = mcolP[:, pidx_[t] * ROWB + i: pidx_[t] * ROWB + i + 1]
                nc.gpsimd.tensor_scalar(dst, mts[t][:, c0:FD], sc, 0.0,
                                        op0=OP.subtract, op1=OP.max)
            else:
                sc = mcolV[:, t * ROWB + i: t * ROWB + i + 1]
                nc.vector.tensor_scalar(dst, mts[t][:, c0:FD], sc, 0.0,
                                        op0=OP.subtract, op1=OP.max)

        def absdiff_odd(t, m, name):
            i = 2 * m + 1
            c0 = 2 * m
            sc = mcolV[:, t * ROWB + i: t * ROWB + i + 1]
            ad = workV.tile([128, FD], dt.bfloat16, tag="adV", name=name)
            nc.vector.tensor_scalar(ad[:, c0:FD], mts[t][:, c0:FD], sc, 0.0,
                                    op0=OP.subtract, op1=OP.max)
            return ad

        # prolog: the first pairs' elementwise is emitted tile-by-tile as
        # the T chunks land, so no engine idles behind another's inputs
        eng_pro = {'dve': PRO_E_DVE, 'pool': PRO_E_POOL, 'act': PRO_E_ACT}
        PRO_E = {t: eng_pro[even_eng[t]] for t in range(NT)}
        PRO_ODD_T = {t: PRO_ODD for t in range(NT)}
        PRO_MAX = max(PRO_ODD, *eng_pro.values())
        f8bufs_m = {m: [workF.tile([128, 2 * FD], dt.float8e4, tag="adF",
                                   name=f"f8_{m}_{p}")
                        for p in range(len(DR_PAIRS))] for m in range(PRO_MAX)}
        ado_m = {m: {} for m in range(PRO_MAX)}

        def prolog(t):
            for m in range(PRO_E[t]):
                absdiff_even(t, m, f8bufs_m[m])
            for m in range(PRO_ODD_T[t]):
                ado_m[m][t] = absdiff_odd(t, m, f"ad_{m}_{t}")

        for n, t in enumerate(order):
            ps = psA.tile([128, FD], dt.float32)
            for kc in range(4):
                nc.tensor.matmul(ps[:],
                                 Tsb[kc][:, t * 128:(t + 1) * 128],
                                 xTsb[kc],
                                 start=(kc == 0), stop=(kc == 3))
            mt_t = mtp.tile([128, FD], dt.bfloat16, tag="mt")
            if t % 4 == 1:
                nc.scalar.copy(mt_t[:], ps[:])
            else:
                nc.vector.tensor_copy(mt_t[:], ps[:])
            mts[t] = mt_t
            # scalar-column copies on the consuming engines
            nc.vector.tensor_copy(mcolV[:, t * ROWB:(t + 1) * ROWB],
                                  mt_t[:, 0:ROWB])
            if t in EVEN_ACT:
                ci = aidx[t]
                nc.scalar.copy(mcolA[:, ci * ROWB:(ci + 1) * ROWB],
                               mt_t[:, 0:ROWB])
            elif t in EVEN_POOL:
                ci = pidx_[t]
                nc.gpsimd.tensor_copy(mcolP[:, ci * ROWB:(ci + 1) * ROWB],
                                      mt_t[:, 0:ROWB])
            if n >= 1:
                prolog(order[n - 1])
            if n >= 2:
                s_matmul(order[n - 2])
        prolog(order[NT - 1])
        s_matmul(order[NT - 2])
        s_matmul(order[NT - 1])

        # S in bf16 (the dmap matmul and the exp bias both derive from
        # S_bf, so the diagonal cancellation is exact)
        S_bf = const.tile([128, FD], dt.bfloat16, tag="S_bf")
        nc.scalar.activation(S_bf[:], psS[:, 0:FD], AF.Copy)
        # exp bias: Sneg2[p, m] = -S_bf[p, 2m + (p // 64)]
        Sneg2 = const.tile([128, NPAIR], dt.float32, tag="Sneg2")
        nc.scalar.activation(Sneg2[0:64, :], S_bf[0:64, 0:2 * NPAIR:2],
                             AF.Copy, scale=-1.0)
        nc.scalar.activation(Sneg2[64:128, :], S_bf[64:128, 1:2 * NPAIR:2],
                             AF.Copy, scale=-1.0)

        psc = psC.tile([OUT_FEAT, 512], dt.float32, tag="psc")

        def exp_pair(psd, m, accum=True):
            c0 = 2 * m
            e_t = ep.tile([128, FD], dt.bfloat16, tag="e", name=f"e_{m}")
            nc.scalar.activation(e_t[:, c0:FD], psd[:, c0:FD], AF.Exp,
                                 scale=-1.0, bias=Sneg2[:, m:m + 1])
            if m < NPAIR - 1:
                nc.sync.dma_start(eS_d[:, m * FD + c0:(m + 1) * FD],
                                  e_t[:, c0:FD])
            return e_t

        def colsum(e_t, m, stop=False):
            c0 = 2 * m + 2
            nc.tensor.matmul(psc[:, c0:FD], sel2, e_t[:, c0:FD],
                             start=False, stop=stop, skip_group_check=True)

        DEFER = 2                # ramp pairs with deferred dmap/exp
        deferred_dmap = []
        pending_exps = []        # (psd, m) awaiting exp
        done_e = {}              # m -> e tile awaiting colsum

        for m in range(NPAIR):
            last = m == NPAIR - 1
            c0 = 2 * m
            # elementwise: DVE's fp8 tiles first so the first DR matmuls
            # have input early; ACT/Pool fill their halves in parallel
            # (prolog pairs were already emitted tile-by-tile above)
            if m < PRO_MAX:
                f8bufs = f8bufs_m[m]
                ado = ado_m[m]
            else:
                f8bufs = [workF.tile([128, 2 * FD], dt.float8e4, tag="adF",
                                     name=f"f8_{m}_{p}")
                          for p in range(len(DR_PAIRS))]
                ado = {}
            for t in EVEN_DVE + EVEN_POOL + EVEN_ACT:
                if m >= PRO_E[t]:
                    absdiff_even(t, m, f8bufs)
            for t in range(NT):
                if m >= PRO_ODD_T[t]:
                    ado[t] = absdiff_odd(t, m, f"ad_{m}_{t}")
            if last:
                # emit exp(30) before the last pair's matmuls: its colsum
                # closes psc mid-stream so the accS tail overlaps pair 31
                psd_e, m_e = pending_exps.pop(0)
                done_e[m_e] = exp_pair(psd_e, m_e)
            if m == 2:
                # open the psc group: pending-zero the bank and write the
                # two never-covered columns (real PSUM powers up dirty)
                nc.tensor.matmul(psc[:, 0:2], zeroW[0:1, 0:OUT_FEAT],
                                 zeroW[0:1, 0:2], start=True, stop=False,
                                 skip_group_check=True)
            for mm in sorted(k for k in done_e if k <= m - 2):
                colsum(done_e.pop(mm), mm)
            # dmap: resets the bank (start=True pending-zeros the 2KB
            # region) and writes -S_half(p)[o(p), j] to all 128 partitions.
            # For the ramp pairs (< DEFER) the reset comes from a cheap K=1
            # zero matmul instead, so the streams can run tile-by-tile as
            # the T chunks land; their dmaps (which need S_bf) and exps are
            # emitted in a batch once S_bf exists, keeping the in-order
            # engine queues free of S_bf waits during the ramp
            psd = psD.tile([128, 512], dt.float32, tag="psd",
                           name=f"psd_{m}")
            if m < DEFER:
                nc.tensor.matmul(psd[:, c0:FD], zeroW[0:1, 0:128],
                                 zrow[0:1, c0:FD], start=True, stop=True)
                deferred_dmap.append((psd, m))
            else:
                if deferred_dmap:
                    for psd_d, m_d in deferred_dmap:
                        nc.tensor.matmul(psd_d[:, 2 * m_d:FD], dmap2,
                                         S_bf[:, 2 * m_d:FD],
                                         start=False, stop=False,
                                         skip_group_check=True)
                    deferred_dmap = []
                    while len(pending_exps) > 1:
                        psd_e, m_e = pending_exps.pop(0)
                        done_e[m_e] = exp_pair(psd_e, m_e)
                nc.tensor.matmul(psd[:, c0:FD], dmap2, S_bf[:, c0:FD],
                                 start=True, stop=True)
            # even half: 8 fp8 DoubleRow matmuls (dst partitions 0:64)
            # odd half: 16 plain 64-wide bf16 matmuls (tile_position (0,64))
            dr_emitted = 0
            dr_order = DR_ORDER
            odd_order = list(range(NT))
            for gi, t in enumerate(odd_order):
                if gi % 2 == 0 and dr_emitted < len(DR_PAIRS):
                    p = dr_order[dr_emitted]
                    dr_emitted += 1
                    w3 = selDR[:, p * 2 * OUT_FEAT:(p + 1) * 2 * OUT_FEAT] \
                        .rearrange("q (two m_) -> q two m_", two=2)
                    x3 = f8bufs[p][:].rearrange("q (two w_) -> q two w_",
                                                two=2)[:, :, c0:FD]
                    nc.tensor.matmul(
                        psd[0:64, c0:FD], w3, x3,
                        start=False, stop=False, skip_group_check=True,
                        perf_mode=mybir.MatmulPerfMode.DoubleRow,
                        tile_position=(0, 0))
                if last and gi == 10:
                    # close the psc group mid-stream: accS copies + DMA
                    # then overlap the rest of pair 31
                    colsum(done_e[m - 1], m - 1, stop=True)
                nc.tensor.matmul(
                    psd[64:128, c0:FD],
                    sel2x[:, t * OUT_FEAT:(t + 1) * OUT_FEAT],
                    ado[t][:, c0:FD],
                    start=False, stop=False, skip_group_check=True,
                    tile_position=(0, 64))
            if last:
                HF = FD // 2
                nc.vector.tensor_copy(accS[:, 0:HF], psc[:, 0:HF])
                nc.vector.tensor_copy(accS[:, HF:FD], psc[:, HF:FD])
                nc.sync.dma_start(acc_d[:], accS[:])
                nc.sync.dma_start(rows_d[:, NPAIR // 2:NPAIR - 1],
                                  rowS2b[:, 0:NPAIR // 2 - 1])
            # exp of the PREVIOUS pair (its PE wait is already satisfied,
            # so ACT never stalls); its colsum is emitted one pair later
            if m >= DEFER and pending_exps:
                psd_e, m_e = pending_exps.pop(0)
                done_e[m_e] = exp_pair(psd_e, m_e)
                if m_e == NPAIR // 2 - 1:
                    nc.sync.dma_start(rows_d[:, 0:NPAIR // 2], rowS2a[:])
            pending_exps.append((psd, m))
        # tail: exp31's tile ships raw; the host folds its rowsums and
        # colsum contributions
        (psd_e, m_e), = pending_exps
        e_last = exp_pair(psd_e, m_e, accum=False)
        nc.sync.dma_start(eL_d[:], e_last[:, 2 * (NPAIR - 1):FD])

    if split_waits:
        _split_multiwaits(nc, mybir)
    return nc


def _split_multiwaits(nc, mybir):
    """Walrus on this toolchain encodes at most ONE sync-wait command per
    instruction.  Split any instruction with more waits into a chain of
    single-wait Drain carriers on the same engine, inserted immediately
    before it."""
    n = 0
    for fn in nc.m.functions:
        for bb in fn.blocks:
            new_insts = []
            for inst in bb.instructions:
                si = getattr(inst, "sync_info", None)
                if si is not None and si.on_wait and len(si.on_wait) > 1:
                    waits = list(si.on_wait)
                    for w in waits[:-1]:
                        carrier = mybir.InstDrain(
                            name=f"splitw_{n}", engine=inst.engine,
                            ins=[], outs=[],
                            sync_info=mybir.SyncInfo(on_wait=[w],
                                                     on_update=[]))
                        new_insts.append(carrier)
                        n += 1
                    inst.sync_info = mybir.SyncInfo(
                        on_wait=[waits[-1]], on_update=list(si.on_update))
                new_insts.append(inst)
            if n:
                bb.instructions = new_insts


def _even_sets():
    even_eng = {}
    for t in EVEN_DVE:
        even_eng[t] = 'dve'
    for t in EVEN_POOL:
        even_eng[t] = 'pool'
    for t in EVEN_ACT:
        even_eng[t] = 'act'
    even_relu = [t for t in range(NT) if even_eng[t] != 'act']
    return even_eng, even_relu


def _selB_host():
    """1.0 weights: tile t partition (g,k) -> output row 4t+g (mod 64)."""
    s = np.zeros((128, NT * OUT_FEAT), dtype=np.float32)
    for t in range(NT):
        for g in range(4):
            s[32 * g:32 * (g + 1), t * OUT_FEAT + (4 * t + g) % OUT_FEAT] = 1.0
    return s.astype(ml_dtypes.bfloat16)


def _sel2x_host():
    s = np.zeros((128, NT * OUT_FEAT), dtype=np.float32)
    for t in range(NT):
        for g in range(4):
            s[32 * g:32 * (g + 1), t * OUT_FEAT + (4 * t + g) % OUT_FEAT] = 2.0
    return s.astype(ml_dtypes.bfloat16)


def _selDR_host():
    """fp8 DoubleRow weights: pair p = (t_lo, t_hi); ktile 0 -> rows of
    t_lo, ktile 1 -> rows of t_hi; 2.0 on relu tiles, 1.0 on ACT tiles."""
    even_eng, _ = _even_sets()
    s = np.zeros((128, len(DR_PAIRS), 2, OUT_FEAT), dtype=np.float32)
    for p, pair in enumerate(DR_PAIRS):
        for half, t in enumerate(pair):
            v = 1.0 if even_eng[t] == 'act' else 2.0
            for g in range(4):
                s[32 * g:32 * (g + 1), p, half, (4 * t + g) % OUT_FEAT] = v
    return s.reshape(128, len(DR_PAIRS) * 2 * OUT_FEAT) \
        .astype(ml_dtypes.float8_e4m3)


def _dmap2_host():
    return (-np.eye(128, dtype=np.float32)).astype(ml_dtypes.bfloat16)


def _sel2_host():
    s = np.zeros((128, OUT_FEAT), dtype=np.float32)
    s[:OUT_FEAT, :] = np.eye(OUT_FEAT)
    s[OUT_FEAT:, :] = np.eye(OUT_FEAT)
    return s.astype(ml_dtypes.bfloat16)


def _block_order(c):
    """Column blocks for core c; None marks the poison block."""
    if c < 4:
        return [c, c + 1, c + 2, c + 3, c + 4]
    return [c, (c + 1) % 8, (c + 2) % 8, (c + 3) % 8, None]


def _in_maps(x, T):
    bf16 = ml_dtypes.bfloat16
    Tb = np.ascontiguousarray(T.reshape(IN_FEAT, OK)).astype(bf16)
    selpk = np.ascontiguousarray(np.concatenate(
        [_selB_host(), _sel2x_host(), _sel2_host(), _dmap2_host()],
        axis=1))
    selDR = _selDR_host()
    xT = np.ascontiguousarray(x.T)
    maps = []
    for c in range(N_CORES):
        xTc = np.empty((IN_FEAT, FD), dtype=np.float32)
        for pos, b in enumerate(_block_order(c)):
            if b is None:
                xTc[:, 64 * pos:64 * (pos + 1)] = POISON
            else:
                xTc[:, 64 * pos:64 * (pos + 1)] = xT[:, 64 * b:64 * (b + 1)]
        maps.append({"xT": xTc.astype(bf16), "Tm": Tb, "selpk": selpk,
                     "selDR": selDR})
    return maps


def _gather(results):
    """results: per-core dict with rowS2 [128, NPAIR], accS [64, FD],
    eLast [128, EL_W] and eShip [128, NPAIR*FD] (raw exp tiles of the
    last + even pairs; rowsums folded here)."""
    EL_C0 = 2 * (NPAIR - 1)      # 62
    mbd = np.zeros((BATCH, OUT_FEAT), dtype=np.float32)
    for c in range(N_CORES):
        rs = np.array(results[c]["rowS2"], dtype=np.float32)
        eL = np.asarray(results[c]["eLast"], dtype=np.float32)  # [128, 258]
        eS = np.asarray(results[c]["eShip"], dtype=np.float32)
        for m in range(NPAIR - 1):
            rs[:, m] = eS[:, m * FD + 2 * m:(m + 1) * FD].sum(axis=1)
        rs[:, NPAIR - 1] = eL.sum(axis=1)
        # partitions [64s:64s+64] of column m are the rowsum of i = 2m+s
        rows = rs.reshape(2, OUT_FEAT, NPAIR).transpose(2, 0, 1)
        mbd[64 * c:64 * (c + 1), :] += rows.reshape(ROWB, OUT_FEAT)
        acc = np.array(results[c]["accS"], dtype=np.float32)  # [o, j]
        # fold the last pair's colsum (cols [64, FD) = eL cols [2:])
        acc[:, EL_C0 + 2:] += eL[0:OUT_FEAT, 2:] + eL[OUT_FEAT:128, 2:]
        for pos, b in enumerate(_block_order(c)):
            if b is None:
                continue
            if pos == 0:
                b = c  # diagonal block: mirror the triangle
            mbd[64 * b:64 * (b + 1), :] += acc[:, 64 * pos:64 * (pos + 1)].T
    mbd -= 1.0  # every rowsum includes its self-term exp(0)=1
    return mbd


def kernel(x, T):
    from concourse import bass_utils

    x = np.asarray(x, dtype=np.float32)
    T = np.asarray(T, dtype=np.float32)

    if "nc" not in _cache:
        _cache["nc"] = _build_nc()
    nc = _cache["nc"]

    res = bass_utils.run_bass_kernel_spmd(
        nc, _in_maps(x, T), core_ids=list(range(N_CORES)))

    mbd = _gather(res.results)
    return np.concatenate([x, mbd], axis=1)
